# revision 44
# baseline (speedup 1.0000x reference)
"""Trainium2 Bass kernel for a sparse-attention EncoderLayer.

Sharding: rows (L) split into 8 contiguous shards of L/8; each edge is owned
by the core that owns its destination row (row_index is sorted, so each
core's edges are a contiguous range).  Each core computes Q/K/V for its row
shard; K/V shards are AllGathered (bf16, in 4 overlapping chunks) so every
core holds the full K/V table in HBM; per-edge K/V and Q rows are fetched
with dma_gather.  Segment softmax runs without max-subtraction (scores are
bounded, exp cannot overflow in f32).  Per-edge one-hot row selectors are
precomputed on the host and DMA'd in; the alpha-weighted scatter and softmax
sums are one-hot PE matmuls accumulated in PSUM per 128-row block.  The
LN2+MLP tail is fused into the edge phase per finished block.

DVE diet relative to the first version: the one-hot build, the p-broadcast
expansion and the x1 HBM roundtrip are gone; per-edge math is batched per
gather chunk (one DVE op per chunk instead of per 128-edge tile).
"""

import math
import numpy as np
from contextlib import ExitStack

from ml_dtypes import bfloat16

import concourse.bass as bass
import concourse.mybir as mybir
import concourse.tile as tile
from concourse import bacc
from concourse.bass_utils import run_bass_kernel_spmd
from concourse.masks import make_identity

NCORES = 8
C, H, D, HID = 512, 8, 64, 1024
EPS = 1e-5
CHUNK_T = 8   # edge tiles (of 128 edges) per dma_gather chunk
NAG = 8       # allgather chunks
F32 = mybir.dt.float32
BF16 = mybir.dt.bfloat16
I16 = mybir.dt.int16
AF = mybir.ActivationFunctionType
ALU = mybir.AluOpType
AX = mybir.AxisListType

_prog_cache = {}
TRACE = False
LAST_EXEC_NS = None
LAST_RESULTS = None


# --------------------------------------------------------------------------
# host-side preprocessing
# --------------------------------------------------------------------------

def _nag(NBLK):
    return NAG if NBLK % NAG == 0 else 1


def _wrap_idx(idx):
    """[n] int -> [128, n//16] int16, wrapped (idx i at partition i%16,
    column i//16) and replicated across the 8 Q7 cores."""
    n = idx.shape[0]
    w = np.ascontiguousarray(idx.reshape(n // 16, 16).T).astype(np.int16)
    return np.tile(w, (8, 1))


def _preprocess_edges(L, row, col, att_bias):
    LSH = L // NCORES
    NBLK = LSH // 128
    bounds = np.searchsorted(row, np.arange(NCORES + 1) * LSH)

    per_core = []
    t_blk = 1
    for c in range(NCORES):
        e0, e1 = int(bounds[c]), int(bounds[c + 1])
        r = row[e0:e1] - c * LSH
        blk = r >> 7
        cnt = np.bincount(blk, minlength=NBLK)
        t_blk = max(t_blk, int(np.max((cnt + 127) // 128)) if len(cnt) else 1)
        per_core.append((e0, e1, r, blk, cnt))

    T_BLK = t_blk
    NT = NBLK * T_BLK
    NCH = (NT + CHUNK_T - 1) // CHUNK_T
    NTP = NCH * CHUNK_T
    LSH4 = LSH // _nag(NBLK)

    cores = []
    for c in range(NCORES):
        e0, e1, r, blk, cnt = per_core[c]
        ne = e1 - e0
        starts = np.zeros(NBLK, dtype=np.int64)
        np.cumsum(cnt[:-1], out=starts[1:])

        npad = NTP * 128
        # col: global node id -> kv_full row (allgather chunk-major layout)
        gcol = col[e0:e1]
        oc, loc = gcol // LSH, gcol % LSH
        kvrow = (loc // LSH4) * (NCORES * LSH4) + oc * LSH4 + (loc % LSH4)
        # order edges within each block by kv row: improves gather locality
        # and lets early chunks depend on only a prefix of the allgather
        perm = np.lexsort((kvrow, blk))
        blk_s = blk[perm]
        kvrow_s = kvrow[perm]
        idx_in_blk = np.arange(ne, dtype=np.int64) - starts[blk_s]
        dst = blk_s * (T_BLK * 128) + idx_in_blk

        colP = np.zeros(npad, dtype=np.int64)
        rlocP = np.zeros(npad, dtype=np.int64)
        biasP = np.full((npad, H), -30000.0, dtype=np.float32)
        colP[dst] = kvrow_s
        rlocP[dst] = r[perm] & 127
        biasP[dst] = att_bias[e0:e1][perm]
        # per-chunk upper bound on referenced kv rows (for partial AG deps)
        maxrow = colP.reshape(NCH, CHUNK_T * 128).max(axis=1) + 1

        colw = _wrap_idx(colP).reshape(128, NCH, CHUNK_T * 8).transpose(1, 0, 2)
        colw = colw.reshape(NCH * 128, CHUNK_T * 8)
        # one-hot row selector per edge, chunk-partition-major for contiguous
        # DMA: ohP[ch, e(part), t_in_ch, r] bf16 via the u16 bit trick.
        # oh[t, e, r]: edge-partition (scatter lhsT); only real edges are set.
        ohu = np.zeros((NTP * 128, 128), dtype=np.uint16)
        ohu[dst, rlocP[dst]] = 0x3F80  # bf16 1.0
        oh = (ohu.view(bfloat16).reshape(NCH, CHUNK_T, 128, 128)
              .transpose(0, 2, 1, 3).reshape(NCH, 128, CHUNK_T * 128))
        # ohT[t, r, e]: row-partition (q-gather lhsT); set for ALL padded
        # slots too (col 0 row 0) so no garbage — padded p is 0 via bias.
        e_in_t = np.arange(npad, dtype=np.int64) % 128
        ohTu = np.zeros((NTP * 128, 128), dtype=np.uint16)
        ohTu[(np.arange(npad) // 128) * 128 + rlocP, e_in_t] = 0x3F80
        ohT = (ohTu.view(bfloat16).reshape(NCH, CHUNK_T, 128, 128)
               .transpose(0, 2, 1, 3).reshape(NCH, 128, CHUNK_T * 128))
        # bias, chunk-partition-major bf16: [NCH, 128, CHUNK_T*H]
        biasT = (biasP.reshape(NCH, CHUNK_T, 128, H).transpose(0, 2, 1, 3)
                 .reshape(NCH, 128, CHUNK_T * H).astype(bfloat16))
        cores.append(dict(
            colw=np.ascontiguousarray(colw),
            biasP=np.ascontiguousarray(biasT),
            ohP=np.ascontiguousarray(oh),
            ohTP=np.ascontiguousarray(ohT),
        ))
        cores[-1]["_maxrow"] = maxrow
    # chunk AG-dep bound must be identical across cores (same program):
    maxrow_all = np.max([c.pop("_maxrow") for c in cores], axis=0)
    return T_BLK, NT, NCH, [int(x) for x in maxrow_all], cores


def _prep_weights(inp):
    scale = 1.0 / math.sqrt(D)

    def mat(w, kchunks):
        w = np.asarray(w, np.float32)
        k, n = w.shape
        assert k == kchunks * 128
        return np.ascontiguousarray(
            w.reshape(kchunks, 128, n).transpose(1, 0, 2)).astype(bfloat16)

    def rowv(b):
        return np.asarray(b, np.float32)[None, :].astype(bfloat16)

    return dict(
        wq=mat(np.asarray(inp["Wq"], np.float32) * scale, 4),
        wk=mat(inp["Wk"], 4),
        wv=mat(inp["Wv"], 4),
        wo=mat(inp["Wo"], 4),
        w1=mat(inp["W1"], 4),
        w2=mat(inp["W2"], 8),
        bq=rowv(np.asarray(inp["bq"], np.float32) * scale),
        bk=rowv(inp["bk"]), bv=rowv(inp["bv"]), bo=rowv(inp["bo"]),
        b1=rowv(inp["b1"]), b2=rowv(inp["b2"]),
        ln1g=np.asarray(inp["ln1_g"], np.float32)[None, :].astype(bfloat16),
        ln1b=np.asarray(inp["ln1_b"], np.float32)[None, :].astype(bfloat16),
        ln2g=np.asarray(inp["ln2_g"], np.float32)[None, :].astype(bfloat16),
        ln2b=np.asarray(inp["ln2_b"], np.float32)[None, :].astype(bfloat16),
    )


# --------------------------------------------------------------------------
# walrus workaround: split Drain instructions carrying >1 sem wait
# --------------------------------------------------------------------------

def _split_multi_waits(nc):
    nid = [0]
    for fn in nc.m.functions:
        for blk in fn.blocks:
            insts = blk.instructions
            i = 0
            while i < len(insts):
                inst = insts[i]
                si = inst.sync_info
                if (isinstance(inst, mybir.InstDrain)
                        and si is not None and si.on_wait and len(si.on_wait) > 1):
                    waits = list(si.on_wait)
                    nops = []
                    for w in waits[:-1]:
                        nid[0] += 1
                        nops.append(mybir.InstNoOp(
                            name=f"I-waitfix-{nid[0]}",
                            engine=inst.engine, ins=[], outs=[],
                            sync_info=mybir.SyncInfo(on_wait=[w], on_update=[]),
                        ))
                    inst.sync_info = mybir.SyncInfo(
                        on_wait=[waits[-1]], on_update=list(si.on_update))
                    insts[i:i] = nops
                    i += len(nops)
                i += 1


# --------------------------------------------------------------------------
# device program
# --------------------------------------------------------------------------

def _build_program(L, T_BLK, NT, NCH, maxrow):
    LSH = L // NCORES
    NBLK = LSH // 128
    nag = _nag(NBLK)
    LSH4 = LSH // nag
    BPA = NBLK // nag  # blocks per allgather chunk
    nc = bacc.Bacc(num_devices=NCORES)

    x_c = nc.declare_dram_parameter("x_c", [LSH, C], F32, isOutput=False)
    wq = nc.declare_dram_parameter("wq", [128, 4, C], BF16, isOutput=False)
    wk = nc.declare_dram_parameter("wk", [128, 4, C], BF16, isOutput=False)
    wv = nc.declare_dram_parameter("wv", [128, 4, C], BF16, isOutput=False)
    wo = nc.declare_dram_parameter("wo", [128, 4, C], BF16, isOutput=False)
    w1 = nc.declare_dram_parameter("w1", [128, 4, HID], BF16, isOutput=False)
    w2 = nc.declare_dram_parameter("w2", [128, 8, C], BF16, isOutput=False)
    bqp = nc.declare_dram_parameter("bq", [1, C], BF16, isOutput=False)
    bkp = nc.declare_dram_parameter("bk", [1, C], BF16, isOutput=False)
    bvp = nc.declare_dram_parameter("bv", [1, C], BF16, isOutput=False)
    bop = nc.declare_dram_parameter("bo", [1, C], BF16, isOutput=False)
    b1p = nc.declare_dram_parameter("b1", [1, HID], BF16, isOutput=False)
    b2p = nc.declare_dram_parameter("b2", [1, C], BF16, isOutput=False)
    ln1g = nc.declare_dram_parameter("ln1g", [1, C], BF16, isOutput=False)
    ln1b = nc.declare_dram_parameter("ln1b", [1, C], BF16, isOutput=False)
    ln2g = nc.declare_dram_parameter("ln2g", [1, C], BF16, isOutput=False)
    ln2b = nc.declare_dram_parameter("ln2b", [1, C], BF16, isOutput=False)
    colw = nc.declare_dram_parameter("colw", [NCH * 128, CHUNK_T * 8], I16, isOutput=False)
    biasP = nc.declare_dram_parameter("biasP", [NCH, 128, CHUNK_T * H], BF16, isOutput=False)
    ohP = nc.declare_dram_parameter("ohP", [NCH, 128, CHUNK_T * 128], BF16, isOutput=False)
    ohTP = nc.declare_dram_parameter("ohTP", [NCH, 128, CHUNK_T * 128], BF16, isOutput=False)
    y_out = nc.declare_dram_parameter("y", [LSH, C], F32, isOutput=True)

    with ExitStack() as ctx:
        tc = ctx.enter_context(tile.TileContext(nc))

        dram = ctx.enter_context(tc.tile_pool(name="dram", bufs=1, space="DRAM"))
        kv_sh = dram.tile([LSH, 2 * C], BF16)
        # chunk-major full table: [NAG][NCORES][LSH4]
        kv_full = dram.tile([NCORES * LSH, 2 * C], BF16)

        # ---------------- constants + weights ----------------
        consts = ctx.enter_context(tc.tile_pool(name="consts", bufs=1))
        ident = consts.tile([128, 128], BF16, tag="ident")
        make_identity(nc, ident[:])
        ones_k1 = consts.tile([1, 128], BF16, tag="ones")
        nc.vector.memset(ones_k1[:], 1.0)
        eps_t = consts.tile([128, 1], F32, tag="eps")
        nc.vector.memset(eps_t[:], EPS)

        def bcast_load(param, tag):
            t = consts.tile([128, C], BF16, tag=tag)
            ap = param[:]
            src = bass.AP(tensor=ap.tensor, offset=ap.offset,
                          ap=[[0, 128], [1, C]])
            nc.sync.dma_start(out=t[:], in_=src)
            return t

        g1_bc, b1_bc = bcast_load(ln1g, "g1"), bcast_load(ln1b, "b1")
        g2_bc, b2_bc = bcast_load(ln2g, "g2"), bcast_load(ln2b, "b2")

        wts = ctx.enter_context(tc.tile_pool(name="wts", bufs=1))

        def wload(p, shape, tag):
            t = wts.tile(shape, BF16, tag=tag)
            nc.sync.dma_start(out=t[:], in_=p[:])
            return t

        wq_sb = wload(wq, [128, 4, C], "wq"); wk_sb = wload(wk, [128, 4, C], "wk")
        wv_sb = wload(wv, [128, 4, C], "wv"); wo_sb = wload(wo, [128, 4, C], "wo")
        w1_sb = wload(w1, [128, 4, HID], "w1"); w2_sb = wload(w2, [128, 8, C], "w2")
        bq_sb = wload(bqp, [1, C], "bq"); bk_sb = wload(bkp, [1, C], "bk")
        bv_sb = wload(bvp, [1, C], "bv"); bo_sb = wload(bop, [1, C], "bo")
        b1_sb = wload(b1p, [1, HID], "bb1"); b2_sb = wload(b2p, [1, C], "bb2")

        # ---------------- LN helper (fused tensor_scalar) ----------------
        def layernorm(pool, lnpool, xb, g_bc, bb_bc, tagp):
            stats = lnpool.tile([128, 6], F32, tag=tagp + "st")
            nc.vector.bn_stats(stats[:], xb[:])
            mv = lnpool.tile([128, 2], F32, tag=tagp + "mv")
            nc.vector.bn_aggr(mv[:], stats[:])
            sd = lnpool.tile([128, 1], F32, tag=tagp + "sd")
            nc.scalar.activation(sd[:], mv[:, 1:2], AF.Sqrt, bias=eps_t[:])
            rstd = lnpool.tile([128, 1], F32, tag=tagp + "rs")
            nc.vector.reciprocal(rstd[:], sd[:])
            z0 = pool.tile([128, C], BF16, tag=tagp + "z0")
            nc.vector.tensor_scalar(z0[:], xb[:], mv[:, 0:1], rstd[:],
                                    op0=ALU.subtract, op1=ALU.mult)
            z1 = pool.tile([128, C], BF16, tag=tagp + "z1")
            nc.vector.tensor_tensor(z1[:], z0[:], g_bc[:], op=ALU.mult)
            zb = pool.tile([128, C], BF16, tag=tagp + "zo")
            nc.vector.tensor_tensor(zb[:], z1[:], bb_bc[:], op=ALU.add)
            return zb

        # q table lives in SBUF for the whole run: [128 rows, NBLK, C]
        qtab_pool = ctx.enter_context(tc.tile_pool(name="qtab", bufs=1))
        q_sb = qtab_pool.tile([128, NBLK, C], BF16)

        # ---------------- phase B: LN1, zT, QKV (+chunked allgather) -------
        # LN1 stats are computed in a first sweep (DVE-only, batched sqrt
        # and reciprocal) so the per-block emission has no DVE<->Act
        # ping-pong on its critical path.
        with ExitStack() as pctx:
            zT_pool = pctx.enter_context(tc.tile_pool(name="zT", bufs=1))
            zT = zT_pool.tile([128, 4, LSH], BF16)
            xp = pctx.enter_context(tc.tile_pool(name="xp", bufs=3))
            lnp = pctx.enter_context(tc.tile_pool(name="lnp", bufs=4))
            trp = pctx.enter_context(tc.tile_pool(name="trp", bufs=2, space="PSUM"))
            qkvp = pctx.enter_context(tc.tile_pool(name="qkvp", bufs=2, space="PSUM"))
            obp = pctx.enter_context(tc.tile_pool(name="obp", bufs=3))

            mvall = zT_pool.tile([128, NBLK, 2], F32, tag="mvall")
            for ib in range(NBLK):
                sl = slice(ib * 128, (ib + 1) * 128)
                xb = xp.tile([128, C], F32, tag="xin")
                nc.sync.dma_start(out=xb[:], in_=x_c[sl, :])
                stats = lnp.tile([128, 6], F32, tag="l1st")
                nc.vector.bn_stats(stats[:], xb[:])
                nc.vector.bn_aggr(mvall[:, ib, :], stats[:])
            sdall = zT_pool.tile([128, NBLK], F32, tag="sdall")
            nc.scalar.activation(sdall[:], mvall[:, :, 1], AF.Sqrt, bias=eps_t[:])
            rsall = zT_pool.tile([128, NBLK], F32, tag="rsall")
            nc.vector.reciprocal(rsall[:], sdall[:])

            for ib in range(NBLK):
                sl = slice(ib * 128, (ib + 1) * 128)
                xb = xp.tile([128, C], F32, tag="xin")
                nc.sync.dma_start(out=xb[:], in_=x_c[sl, :])
                z0 = xp.tile([128, C], BF16, tag="l1z0")
                nc.vector.tensor_scalar(z0[:], xb[:], mvall[:, ib, 0:1],
                                        rsall[:, ib:ib + 1],
                                        op0=ALU.subtract, op1=ALU.mult)
                z1 = xp.tile([128, C], BF16, tag="l1z1")
                nc.vector.tensor_tensor(z1[:], z0[:], g1_bc[:], op=ALU.mult)
                zb = xp.tile([128, C], BF16, tag="l1zo")
                nc.vector.tensor_tensor(zb[:], z1[:], b1_bc[:], op=ALU.add)
                for g in range(4):
                    pt = trp.tile([128, 128], BF16)
                    nc.tensor.transpose(pt[:], zb[:, g * 128:(g + 1) * 128], ident[:])
                    nc.scalar.copy(zT[:, g, sl], pt[:])
                for w_sb, bias_sb, dst in (
                    (wq_sb, bq_sb, None),
                    (wk_sb, bk_sb, 0),
                    (wv_sb, bv_sb, 1),
                ):
                    ps = qkvp.tile([128, C], F32)
                    for g in range(4):
                        nc.tensor.matmul(ps[:], lhsT=zT[:, g, sl], rhs=w_sb[:, g, :],
                                         start=(g == 0), stop=False)
                    nc.tensor.matmul(ps[:], lhsT=ones_k1[:], rhs=bias_sb[:],
                                     start=False, stop=True)
                    if dst is None:
                        nc.scalar.copy(q_sb[:, ib, :], ps[:])
                    else:
                        ob = obp.tile([128, C], BF16)
                        nc.scalar.copy(ob[:], ps[:])
                        nc.sync.dma_start(out=kv_sh[sl, dst * C:(dst + 1) * C], in_=ob[:])
                # fire allgather for each finished quarter
                if (ib + 1) % BPA == 0:
                    j = (ib + 1) // BPA - 1
                    nc.gpsimd.collective_compute(
                        "AllGather", ALU.bypass,
                        replica_groups=[list(range(NCORES))],
                        ins=[kv_sh[j * LSH4:(j + 1) * LSH4, :]],
                        outs=[kv_full[j * NCORES * LSH4:(j + 1) * NCORES * LSH4, :]],
                    )

        # ---------------- phase E: edges + fused per-block tail ----------
        # Software-pipelined emission: per iteration ch we emit
        #   stage1(ch):  qps matmuls + prods + reduce + bias        [PE/DVE]
        #   wt(ch-1)                                                [DVE]
        #   dma(ch+1) prefetch                                      [Sync/Q7]
        #   acts(ch):   exp8 + expand                               [Act]
        #   scatter(ch-1) + block tails                             [PE/...]
        # so no engine stream head-of-line-blocks on another engine's
        # freshly-queued work.
        with ExitStack() as pctx:
            kvp = pctx.enter_context(tc.tile_pool(name="kvp", bufs=3))
            idxp = pctx.enter_context(tc.tile_pool(name="idxp", bufs=3))
            bp = pctx.enter_context(tc.tile_pool(name="bp", bufs=2))
            ohp_ = pctx.enter_context(tc.tile_pool(name="ohp", bufs=4))
            ohtp = pctx.enter_context(tc.tile_pool(name="ohtp", bufs=2))
            workp = pctx.enter_context(tc.tile_pool(name="workp", bufs=1))
            work = pctx.enter_context(tc.tile_pool(name="work", bufs=2))
            work3 = pctx.enter_context(tc.tile_pool(name="work3", bufs=3))
            pop_ = pctx.enter_context(tc.tile_pool(name="pout", bufs=1, space="PSUM"))
            mmp = pctx.enter_context(tc.tile_pool(name="mm512", bufs=5, space="PSUM"))
            trp2 = pctx.enter_context(tc.tile_pool(name="trp2", bufs=1, space="PSUM"))
            finp = pctx.enter_context(tc.tile_pool(name="finp", bufs=1))
            lnp2 = pctx.enter_context(tc.tile_pool(name="lnp2", bufs=2))

            def _block_tail(rb_, pout, _unused):
                # ---- fused block tail: att, Wo, residual, LN2, MLP ----
                sl = slice(rb_ * 128, (rb_ + 1) * 128)
                sm = finp.tile([128, H], F32, tag="sm")
                nc.vector.tensor_scalar(sm[:], pout[:, C:C + H], 1e-30, None, op0=ALU.max)
                rec = finp.tile([128, H], F32, tag="rec")
                nc.vector.reciprocal(rec[:], sm[:])
                rexp = finp.tile([128, C], BF16, tag="rexp")
                rap = bass.AP(tensor=rec.tensor, offset=rec[:].offset,
                              ap=[rec[:].ap[0], [1, H], [0, D]])
                nc.scalar.activation(
                    rexp[:].rearrange("p (h d) -> p h d", h=H), rap, AF.Copy)
                att = finp.tile([128, C], BF16, tag="att")
                nc.vector.tensor_tensor(att[:], pout[:, 0:C], rexp[:], op=ALU.mult)
                attT = finp.tile([128, 4, 128], BF16, tag="attT")
                for g in range(4):
                    pt = trp2.tile([128, 128], BF16)
                    nc.tensor.transpose(pt[:], att[:, g * 128:(g + 1) * 128], ident[:])
                    nc.scalar.copy(attT[:, g, :], pt[:])
                po = mmp.tile([128, C], F32, tag="mm")
                for g in range(4):
                    nc.tensor.matmul(po[:], lhsT=attT[:, g, :], rhs=wo_sb[:, g, :],
                                     start=(g == 0), stop=False)
                nc.tensor.matmul(po[:], lhsT=ones_k1[:], rhs=bo_sb[:],
                                 start=False, stop=True)
                xb2 = finp.tile([128, C], F32, tag="xb2")
                nc.sync.dma_start(out=xb2[:], in_=x_c[sl, :])
                x1t = finp.tile([128, C], F32, tag="x1t")
                nc.vector.tensor_tensor(x1t[:], po[:], xb2[:], op=ALU.add)
                # LN2 + MLP
                z2 = layernorm(finp, lnp2, x1t, g2_bc, b2_bc, "l2")
                z2T = finp.tile([128, 4, 128], BF16, tag="z2T")
                for g in range(4):
                    pt = trp2.tile([128, 128], BF16)
                    nc.tensor.transpose(pt[:], z2[:, g * 128:(g + 1) * 128], ident[:])
                    nc.scalar.copy(z2T[:, g, :], pt[:])
                hs = finp.tile([128, 8, 128], BF16, tag="hs")
                for half in range(2):
                    ph_t = mmp.tile([128, C], F32, tag="mm")
                    ph = ph_t[:].rearrange("p (a b) -> p a b", a=4)
                    for c4 in range(4):
                        chc = half * 4 + c4
                        csl = slice(chc * 128, (chc + 1) * 128)
                        for g in range(4):
                            nc.tensor.matmul(ph[:, c4, :], lhsT=w1_sb[:, g, csl],
                                             rhs=z2T[:, g, :], start=(g == 0), stop=False)
                        nc.tensor.matmul(ph[:, c4, :], lhsT=b1_sb[:, csl],
                                         rhs=ones_k1[:], start=False, stop=True)
                    nc.scalar.activation(hs[:, half * 4:(half + 1) * 4, :], ph[:, :, :], AF.Silu)
                py = mmp.tile([128, C], F32, tag="mm")
                for chc in range(8):
                    nc.tensor.matmul(py[:], lhsT=hs[:, chc, :], rhs=w2_sb[:, chc, :],
                                     start=(chc == 0), stop=False)
                nc.tensor.matmul(py[:], lhsT=ones_k1[:], rhs=b2_sb[:],
                                 start=False, stop=True)
                yt = finp.tile([128, C], F32, tag="yt")
                nc.vector.tensor_tensor(yt[:], py[:], x1t[:], op=ALU.add)
                nc.sync.dma_start(out=y_out[sl, :], in_=yt[:])

            state = {"pout": None, "pssum": None}
            stash = {}

            def _emit_dma(ch):
                tiles_c = min(CHUNK_T, NT - ch * CHUNK_T)
                n_idx = tiles_c * 128
                cidx = idxp.tile([128, CHUNK_T * 8], I16, tag="cidx")
                nc.sync.dma_start(out=cidx[:], in_=colw[ch * 128:(ch + 1) * 128, :])
                kvb = kvp.tile([128, CHUNK_T, 2 * C], BF16)
                nc.gpsimd.dma_gather(
                    out_ap=kvb[:, :tiles_c, :], in_ap=kv_full[0:maxrow[ch], :],
                    idxs_ap=cidx[:, :n_idx // 16],
                    num_idxs=n_idx, num_idxs_reg=n_idx, elem_size=2 * C,
                    single_packet=False)
                bia = bp.tile([128, CHUNK_T, H], BF16, tag="bia")
                nc.sync.dma_start(
                    out=bia[:, :tiles_c, :],
                    in_=biasP[ch, :, :tiles_c * H].rearrange(
                        "p (t h) -> p t h", h=H))
                ohc = ohp_.tile([128, CHUNK_T, 128], BF16, tag="oh")
                nc.sync.dma_start(
                    out=ohc[:, :tiles_c, :],
                    in_=ohP[ch, :, :tiles_c * 128].rearrange(
                        "p (t r) -> p t r", r=128))
                ohtc = ohtp.tile([128, CHUNK_T, 128], BF16, tag="ohT")
                nc.sync.dma_start(
                    out=ohtc[:, :tiles_c, :],
                    in_=ohTP[ch, :, :tiles_c * 128].rearrange(
                        "p (t e) -> p t e", e=128))
                return dict(tiles_c=tiles_c, kvb=kvb, bia=bia, ohc=ohc, ohtc=ohtc)

            def _emit_stage1(ch, dd):
                tc_ = dd["tiles_c"]
                prod = workp.tile([128, CHUNK_T, C], BF16, tag="prod")
                for slot in range(tc_):
                    t = ch * CHUNK_T + slot
                    rb = t // T_BLK
                    qps = mmp.tile([128, C], F32, tag="mm")
                    nc.tensor.matmul(qps[:], lhsT=dd["ohtc"][:, slot, :],
                                     rhs=q_sb[:, rb, :], start=True, stop=True)
                    nc.vector.tensor_tensor(prod[:, slot, :], dd["kvb"][:, slot, 0:C],
                                            qps[:], op=ALU.mult)
                sc = work.tile([128, CHUNK_T, H], F32, tag="sc")
                nc.vector.tensor_reduce(
                    sc[:, :tc_, :],
                    prod[:, :tc_, :].rearrange("p t (h d) -> p t h d", h=H),
                    axis=AX.X, op=ALU.add)
                sc2 = work.tile([128, CHUNK_T, H], F32, tag="sc2")
                nc.vector.tensor_tensor(sc2[:, :tc_, :], sc[:, :tc_, :],
                                        dd["bia"][:, :tc_, :], op=ALU.add)
                dd["sc2"] = sc2

            def _emit_acts(ch, dd):
                tc_ = dd["tiles_c"]
                sc2 = dd["sc2"]
                p8c = work3.tile([128, CHUNK_T, H], BF16, tag="p8")
                nc.scalar.activation(p8c[:, :tc_, :], sc2[:, :tc_, :], AF.Exp)
                dd["p8c"] = p8c
                wtc = work3.tile([128, CHUNK_T, C], BF16, tag="wt")
                pexp = work.tile([128, CHUNK_T, C], BF16, tag="pexp")
                s2 = sc2[:, :tc_, :]
                src_b = bass.AP(tensor=s2.tensor, offset=s2.offset,
                                ap=[s2.ap[0], s2.ap[1], s2.ap[2], [0, D]])
                nc.scalar.activation(
                    pexp[:, :tc_, :].rearrange("p t (h d) -> p t h d", h=H),
                    src_b, AF.Exp)
                dd["wtc"] = wtc
                dd["pexp"] = pexp

            def _emit_wt(ch, dd):
                tc_ = dd["tiles_c"]
                wtc = dd["wtc"]
                nc.vector.tensor_tensor(wtc[:, :tc_, 0:C], dd["kvb"][:, :tc_, C:2 * C],
                                        dd["pexp"][:, :tc_, :], op=ALU.mult)

            def _emit_scatter(ch, dd):
                for s in range(dd["tiles_c"]):
                    ts_ = ch * CHUNK_T + s
                    rb_, tb_ = divmod(ts_, T_BLK)
                    if tb_ == 0:
                        state["pout"] = pop_.tile([128, C + H], F32, tag="pout", name="pout")
                    nc.tensor.matmul(state["pout"][:, 0:C], lhsT=dd["ohc"][:, s, :],
                                     rhs=dd["wtc"][:, s, :],
                                     start=(tb_ == 0), stop=(tb_ == T_BLK - 1))
                    nc.tensor.matmul(state["pout"][:, C:C + H], lhsT=dd["ohc"][:, s, :],
                                     rhs=dd["p8c"][:, s, :],
                                     start=(tb_ == 0), stop=(tb_ == T_BLK - 1))
                    if tb_ == T_BLK - 1:
                        _block_tail(rb_, state["pout"], None)

            # 3-stage pipeline: stage1(ch) | wt(ch-1) | scatter(ch-2)
            stash[0] = _emit_dma(0)
            for ch in range(NCH):
                _emit_stage1(ch, stash[ch])
                if ch >= 1:
                    _emit_wt(ch - 1, stash[ch - 1])
                if ch + 1 < NCH:
                    stash[ch + 1] = _emit_dma(ch + 1)
                _emit_acts(ch, stash[ch])
                if ch >= 2:
                    _emit_scatter(ch - 2, stash[ch - 2])
                    del stash[ch - 2]
            _emit_wt(NCH - 1, stash[NCH - 1])
            _emit_scatter(NCH - 2, stash[NCH - 2])
            _emit_scatter(NCH - 1, stash[NCH - 1])

    nc.finalize()
    _split_multi_waits(nc)
    return nc


# --------------------------------------------------------------------------
# entry point
# --------------------------------------------------------------------------

def kernel(**inputs) -> np.ndarray:
    x = np.asarray(inputs["x"], np.float32)
    row = np.asarray(inputs["row_index"]).astype(np.int64)
    col = np.asarray(inputs["col_index"]).astype(np.int64)
    att_bias = np.asarray(inputs["att_bias"], np.float32)
    L = x.shape[0]
    LSH = L // NCORES

    T_BLK, NT, NCH, maxrow, cores = _preprocess_edges(L, row, col, att_bias)
    # quantize AG-dep bounds to allgather chunk granularity for caching
    S = max(1, L // max(1, _nag(L // NCORES // 128)))
    maxrow = [min(L, -(-m // S) * S) for m in maxrow]

    w = _prep_weights(inputs)

    key = (L, T_BLK, NT, NCH, tuple(maxrow))
    if key not in _prog_cache:
        _prog_cache[key] = _build_program(L, T_BLK, NT, NCH, maxrow)
    nc = _prog_cache[key]

    in_maps = []
    for c in range(NCORES):
        m = dict(w)
        m["x_c"] = np.ascontiguousarray(x[c * LSH:(c + 1) * LSH])
        m.update(cores[c])
        in_maps.append(m)

    global LAST_EXEC_NS, LAST_RESULTS
    res = run_bass_kernel_spmd(nc, in_maps, list(range(NCORES)), trace=TRACE)
    LAST_RESULTS = res
    LAST_EXEC_NS = res.exec_time_ns
    return np.concatenate([res.results[c]["y"] for c in range(NCORES)], axis=0)


# revision 48
# speedup vs baseline: 1.1338x; 1.1338x over previous
"""Trainium2 Bass kernel for a sparse-attention EncoderLayer.

Sharding: rows (L) split into 8 contiguous shards of L/8; each edge is owned
by the core that owns its destination row (row_index is sorted, so each
core's edges are a contiguous range).  Each core computes Q/K/V for its row
shard; K/V shards are AllGathered (bf16, in 4 overlapping chunks) so every
core holds the full K/V table in HBM; per-edge K/V and Q rows are fetched
with dma_gather.  Segment softmax runs without max-subtraction (scores are
bounded, exp cannot overflow in f32).  Per-edge one-hot row selectors are
precomputed on the host and DMA'd in; the alpha-weighted scatter and softmax
sums are one-hot PE matmuls accumulated in PSUM per 128-row block.  The
LN2+MLP tail is fused into the edge phase per finished block.

DVE diet relative to the first version: the one-hot build, the p-broadcast
expansion and the x1 HBM roundtrip are gone; per-edge math is batched per
gather chunk (one DVE op per chunk instead of per 128-edge tile).
"""

import math
import numpy as np
from contextlib import ExitStack

from ml_dtypes import bfloat16

import concourse.bass as bass
import concourse.mybir as mybir
import concourse.tile as tile
from concourse import bacc
from concourse.bass_utils import run_bass_kernel_spmd
from concourse.masks import make_identity

NCORES = 8
C, H, D, HID = 512, 8, 64, 1024
EPS = 1e-5
CHUNK_T = 8   # edge tiles (of 128 edges) per dma_gather chunk
NAG = 8       # allgather chunks
F32 = mybir.dt.float32
BF16 = mybir.dt.bfloat16
I16 = mybir.dt.int16
AF = mybir.ActivationFunctionType
ALU = mybir.AluOpType
AX = mybir.AxisListType

_prog_cache = {}
TRACE = False
LAST_EXEC_NS = None
LAST_RESULTS = None


# --------------------------------------------------------------------------
# host-side preprocessing
# --------------------------------------------------------------------------

def _nag(NBLK):
    return NAG if NBLK % NAG == 0 else 1


def _wrap_idx(idx):
    """[n] int -> [128, n//16] int16, wrapped (idx i at partition i%16,
    column i//16) and replicated across the 8 Q7 cores."""
    n = idx.shape[0]
    w = np.ascontiguousarray(idx.reshape(n // 16, 16).T).astype(np.int16)
    return np.tile(w, (8, 1))


def _preprocess_edges(L, row, col, att_bias):
    LSH = L // NCORES
    NBLK = LSH // 128
    bounds = np.searchsorted(row, np.arange(NCORES + 1) * LSH)

    per_core = []
    t_blk = 1
    for c in range(NCORES):
        e0, e1 = int(bounds[c]), int(bounds[c + 1])
        r = row[e0:e1] - c * LSH
        blk = r >> 7
        cnt = np.bincount(blk, minlength=NBLK)
        t_blk = max(t_blk, int(np.max((cnt + 127) // 128)) if len(cnt) else 1)
        per_core.append((e0, e1, r, blk, cnt))

    T_BLK = t_blk
    NT = NBLK * T_BLK
    NCH = (NT + CHUNK_T - 1) // CHUNK_T
    NTP = NCH * CHUNK_T
    LSH4 = LSH // _nag(NBLK)

    cores = []
    for c in range(NCORES):
        e0, e1, r, blk, cnt = per_core[c]
        ne = e1 - e0
        starts = np.zeros(NBLK, dtype=np.int64)
        np.cumsum(cnt[:-1], out=starts[1:])

        npad = NTP * 128
        # col: global node id -> kv_full row (allgather chunk-major layout)
        gcol = col[e0:e1]
        oc, loc = gcol // LSH, gcol % LSH
        kvrow = (loc // LSH4) * (NCORES * LSH4) + oc * LSH4 + (loc % LSH4)
        # order edges within each block by kv row: improves gather locality
        # and lets early chunks depend on only a prefix of the allgather
        perm = np.lexsort((kvrow, blk))
        blk_s = blk[perm]
        kvrow_s = kvrow[perm]
        idx_in_blk = np.arange(ne, dtype=np.int64) - starts[blk_s]
        dst = blk_s * (T_BLK * 128) + idx_in_blk

        colP = np.zeros(npad, dtype=np.int64)
        rlocP = np.zeros(npad, dtype=np.int64)
        biasP = np.full((npad, H), -30000.0, dtype=np.float32)
        colP[dst] = kvrow_s
        rlocP[dst] = r[perm] & 127
        biasP[dst] = att_bias[e0:e1][perm]
        # per-chunk upper bound on referenced kv rows (for partial AG deps)
        maxrow = colP.reshape(NCH, CHUNK_T * 128).max(axis=1) + 1

        colw = _wrap_idx(colP).reshape(128, NCH, CHUNK_T * 8).transpose(1, 0, 2)
        colw = colw.reshape(NCH * 128, CHUNK_T * 8)
        # one-hot row selector per edge, chunk-partition-major for contiguous
        # DMA: ohP[ch, e(part), t_in_ch, r] bf16 via the u16 bit trick.
        # oh[t, e, r]: edge-partition (scatter lhsT); only real edges are set.
        ohu = np.zeros((NTP * 128, 128), dtype=np.uint16)
        ohu[dst, rlocP[dst]] = 0x3F80  # bf16 1.0
        oh = (ohu.view(bfloat16).reshape(NCH, CHUNK_T, 128, 128)
              .transpose(0, 2, 1, 3).reshape(NCH, 128, CHUNK_T * 128))
        # ohT[t, r, e]: row-partition (q-gather lhsT); set for ALL padded
        # slots too (col 0 row 0) so no garbage — padded p is 0 via bias.
        e_in_t = np.arange(npad, dtype=np.int64) % 128
        ohTu = np.zeros((NTP * 128, 128), dtype=np.uint16)
        ohTu[(np.arange(npad) // 128) * 128 + rlocP, e_in_t] = 0x3F80
        ohT = (ohTu.view(bfloat16).reshape(NCH, CHUNK_T, 128, 128)
               .transpose(0, 2, 1, 3).reshape(NCH, 128, CHUNK_T * 128))
        # bias, chunk-partition-major bf16: [NCH, 128, CHUNK_T*H]
        biasT = (biasP.reshape(NCH, CHUNK_T, 128, H).transpose(0, 2, 1, 3)
                 .reshape(NCH, 128, CHUNK_T * H).astype(bfloat16))
        cores.append(dict(
            colw=np.ascontiguousarray(colw),
            biasP=np.ascontiguousarray(biasT),
            ohP=np.ascontiguousarray(oh),
            ohTP=np.ascontiguousarray(ohT),
        ))
        cores[-1]["_maxrow"] = maxrow
    # chunk AG-dep bound must be identical across cores (same program):
    maxrow_all = np.max([c.pop("_maxrow") for c in cores], axis=0)
    return T_BLK, NT, NCH, [int(x) for x in maxrow_all], cores


def _prep_weights(inp):
    scale = 1.0 / math.sqrt(D)

    def mat(w, kchunks):
        w = np.asarray(w, np.float32)
        k, n = w.shape
        assert k == kchunks * 128
        return np.ascontiguousarray(
            w.reshape(kchunks, 128, n).transpose(1, 0, 2)).astype(bfloat16)

    def rowv(b):
        return np.asarray(b, np.float32)[None, :].astype(bfloat16)

    return dict(
        wq=mat(np.asarray(inp["Wq"], np.float32) * scale, 4),
        wk=mat(inp["Wk"], 4),
        wv=mat(inp["Wv"], 4),
        wo=mat(inp["Wo"], 4),
        w1=mat(inp["W1"], 4),
        w2=mat(inp["W2"], 8),
        bq=rowv(np.asarray(inp["bq"], np.float32) * scale),
        bk=rowv(inp["bk"]), bv=rowv(inp["bv"]), bo=rowv(inp["bo"]),
        b1=rowv(inp["b1"]), b2=rowv(inp["b2"]),
        ln1g=np.asarray(inp["ln1_g"], np.float32)[None, :].astype(bfloat16),
        ln1b=np.asarray(inp["ln1_b"], np.float32)[None, :].astype(bfloat16),
        ln2g=np.asarray(inp["ln2_g"], np.float32)[None, :].astype(bfloat16),
        ln2b=np.asarray(inp["ln2_b"], np.float32)[None, :].astype(bfloat16),
    )


# --------------------------------------------------------------------------
# walrus workaround: split Drain instructions carrying >1 sem wait
# --------------------------------------------------------------------------

def _split_multi_waits(nc):
    nid = [0]
    for fn in nc.m.functions:
        for blk in fn.blocks:
            insts = blk.instructions
            i = 0
            while i < len(insts):
                inst = insts[i]
                si = inst.sync_info
                if (isinstance(inst, mybir.InstDrain)
                        and si is not None and si.on_wait and len(si.on_wait) > 1):
                    waits = list(si.on_wait)
                    nops = []
                    for w in waits[:-1]:
                        nid[0] += 1
                        nops.append(mybir.InstNoOp(
                            name=f"I-waitfix-{nid[0]}",
                            engine=inst.engine, ins=[], outs=[],
                            sync_info=mybir.SyncInfo(on_wait=[w], on_update=[]),
                        ))
                    inst.sync_info = mybir.SyncInfo(
                        on_wait=[waits[-1]], on_update=list(si.on_update))
                    insts[i:i] = nops
                    i += len(nops)
                i += 1


# --------------------------------------------------------------------------
# device program
# --------------------------------------------------------------------------

def _build_program(L, T_BLK, NT, NCH, maxrow):
    LSH = L // NCORES
    NBLK = LSH // 128
    nag = _nag(NBLK)
    LSH4 = LSH // nag
    BPA = NBLK // nag  # blocks per allgather chunk
    nc = bacc.Bacc(num_devices=NCORES)

    x_c = nc.declare_dram_parameter("x_c", [LSH, C], F32, isOutput=False)
    wq = nc.declare_dram_parameter("wq", [128, 4, C], BF16, isOutput=False)
    wk = nc.declare_dram_parameter("wk", [128, 4, C], BF16, isOutput=False)
    wv = nc.declare_dram_parameter("wv", [128, 4, C], BF16, isOutput=False)
    wo = nc.declare_dram_parameter("wo", [128, 4, C], BF16, isOutput=False)
    w1 = nc.declare_dram_parameter("w1", [128, 4, HID], BF16, isOutput=False)
    w2 = nc.declare_dram_parameter("w2", [128, 8, C], BF16, isOutput=False)
    bqp = nc.declare_dram_parameter("bq", [1, C], BF16, isOutput=False)
    bkp = nc.declare_dram_parameter("bk", [1, C], BF16, isOutput=False)
    bvp = nc.declare_dram_parameter("bv", [1, C], BF16, isOutput=False)
    bop = nc.declare_dram_parameter("bo", [1, C], BF16, isOutput=False)
    b1p = nc.declare_dram_parameter("b1", [1, HID], BF16, isOutput=False)
    b2p = nc.declare_dram_parameter("b2", [1, C], BF16, isOutput=False)
    ln1g = nc.declare_dram_parameter("ln1g", [1, C], BF16, isOutput=False)
    ln1b = nc.declare_dram_parameter("ln1b", [1, C], BF16, isOutput=False)
    ln2g = nc.declare_dram_parameter("ln2g", [1, C], BF16, isOutput=False)
    ln2b = nc.declare_dram_parameter("ln2b", [1, C], BF16, isOutput=False)
    colw = nc.declare_dram_parameter("colw", [NCH * 128, CHUNK_T * 8], I16, isOutput=False)
    biasP = nc.declare_dram_parameter("biasP", [NCH, 128, CHUNK_T * H], BF16, isOutput=False)
    ohP = nc.declare_dram_parameter("ohP", [NCH, 128, CHUNK_T * 128], BF16, isOutput=False)
    ohTP = nc.declare_dram_parameter("ohTP", [NCH, 128, CHUNK_T * 128], BF16, isOutput=False)
    y_out = nc.declare_dram_parameter("y", [LSH, C], F32, isOutput=True)

    with ExitStack() as ctx:
        tc = ctx.enter_context(tile.TileContext(nc))

        dram = ctx.enter_context(tc.tile_pool(name="dram", bufs=1, space="DRAM"))
        kv_sh = dram.tile([LSH, 2 * C], BF16)
        # chunk-major full table: [NAG][NCORES][LSH4]
        kv_full = dram.tile([NCORES * LSH, 2 * C], BF16)

        # ---------------- constants + weights ----------------
        consts = ctx.enter_context(tc.tile_pool(name="consts", bufs=1))
        ident = consts.tile([128, 128], BF16, tag="ident")
        make_identity(nc, ident[:])
        ones_k1 = consts.tile([1, 128], BF16, tag="ones")
        nc.vector.memset(ones_k1[:], 1.0)
        eps_t = consts.tile([128, 1], F32, tag="eps")
        nc.vector.memset(eps_t[:], EPS)

        def bcast_load(param, tag):
            t = consts.tile([128, C], BF16, tag=tag)
            ap = param[:]
            src = bass.AP(tensor=ap.tensor, offset=ap.offset,
                          ap=[[0, 128], [1, C]])
            nc.sync.dma_start(out=t[:], in_=src)
            return t

        g1_bc, b1_bc = bcast_load(ln1g, "g1"), bcast_load(ln1b, "b1")
        g2_bc, b2_bc = bcast_load(ln2g, "g2"), bcast_load(ln2b, "b2")

        wts = ctx.enter_context(tc.tile_pool(name="wts", bufs=1))

        def wload(p, shape, tag):
            t = wts.tile(shape, BF16, tag=tag)
            nc.sync.dma_start(out=t[:], in_=p[:])
            return t

        wq_sb = wload(wq, [128, 4, C], "wq"); wk_sb = wload(wk, [128, 4, C], "wk")
        wv_sb = wload(wv, [128, 4, C], "wv"); wo_sb = wload(wo, [128, 4, C], "wo")
        w1_sb = wload(w1, [128, 4, HID], "w1"); w2_sb = wload(w2, [128, 8, C], "w2")
        bq_sb = wload(bqp, [1, C], "bq"); bk_sb = wload(bkp, [1, C], "bk")
        bv_sb = wload(bvp, [1, C], "bv"); bo_sb = wload(bop, [1, C], "bo")
        b1_sb = wload(b1p, [1, HID], "bb1"); b2_sb = wload(b2p, [1, C], "bb2")

        # ---------------- LN helper (fused tensor_scalar) ----------------
        def layernorm(pool, lnpool, xb, g_bc, bb_bc, tagp):
            stats = lnpool.tile([128, 6], F32, tag=tagp + "st")
            nc.vector.bn_stats(stats[:], xb[:])
            mv = lnpool.tile([128, 2], F32, tag=tagp + "mv")
            nc.vector.bn_aggr(mv[:], stats[:])
            sd = lnpool.tile([128, 1], F32, tag=tagp + "sd")
            nc.scalar.activation(sd[:], mv[:, 1:2], AF.Sqrt, bias=eps_t[:])
            rstd = lnpool.tile([128, 1], F32, tag=tagp + "rs")
            nc.vector.reciprocal(rstd[:], sd[:])
            z0 = pool.tile([128, C], BF16, tag=tagp + "z0")
            nc.vector.tensor_scalar(z0[:], xb[:], mv[:, 0:1], rstd[:],
                                    op0=ALU.subtract, op1=ALU.mult)
            z1 = pool.tile([128, C], BF16, tag=tagp + "z1")
            nc.vector.tensor_tensor(z1[:], z0[:], g_bc[:], op=ALU.mult)
            zb = pool.tile([128, C], BF16, tag=tagp + "zo")
            nc.vector.tensor_tensor(zb[:], z1[:], bb_bc[:], op=ALU.add)
            return zb

        # q table lives in SBUF for the whole run: [128 rows, NBLK, C]
        qtab_pool = ctx.enter_context(tc.tile_pool(name="qtab", bufs=1))
        q_sb = qtab_pool.tile([128, NBLK, C], BF16)

        # ---------------- phase B: LN1, zT, QKV (+chunked allgather) -------
        # LN1 stats are computed in a first sweep (DVE-only, batched sqrt
        # and reciprocal) so the per-block emission has no DVE<->Act
        # ping-pong on its critical path.
        with ExitStack() as pctx:
            zT_pool = pctx.enter_context(tc.tile_pool(name="zT", bufs=1))
            zT = zT_pool.tile([128, 4, LSH], BF16)
            xp = pctx.enter_context(tc.tile_pool(name="xp", bufs=3))
            lnp = pctx.enter_context(tc.tile_pool(name="lnp", bufs=4))
            trp = pctx.enter_context(tc.tile_pool(name="trp", bufs=2, space="PSUM"))
            qkvp = pctx.enter_context(tc.tile_pool(name="qkvp", bufs=2, space="PSUM"))
            obp = pctx.enter_context(tc.tile_pool(name="obp", bufs=3))

            mvall = zT_pool.tile([128, NBLK, 2], F32, tag="mvall")
            for ib in range(NBLK):
                sl = slice(ib * 128, (ib + 1) * 128)
                xb = xp.tile([128, C], F32, tag="xin")
                nc.sync.dma_start(out=xb[:], in_=x_c[sl, :])
                stats = lnp.tile([128, 6], F32, tag="l1st")
                nc.vector.bn_stats(stats[:], xb[:])
                nc.vector.bn_aggr(mvall[:, ib, :], stats[:])
            sdall = zT_pool.tile([128, NBLK], F32, tag="sdall")
            nc.scalar.activation(sdall[:], mvall[:, :, 1], AF.Sqrt, bias=eps_t[:])
            rsall = zT_pool.tile([128, NBLK], F32, tag="rsall")
            nc.vector.reciprocal(rsall[:], sdall[:])

            for ib in range(NBLK):
                sl = slice(ib * 128, (ib + 1) * 128)
                xb = xp.tile([128, C], F32, tag="xin")
                nc.sync.dma_start(out=xb[:], in_=x_c[sl, :])
                z0 = xp.tile([128, C], BF16, tag="l1z0")
                nc.vector.tensor_scalar(z0[:], xb[:], mvall[:, ib, 0:1],
                                        rsall[:, ib:ib + 1],
                                        op0=ALU.subtract, op1=ALU.mult)
                z1 = xp.tile([128, C], BF16, tag="l1z1")
                nc.vector.tensor_tensor(z1[:], z0[:], g1_bc[:], op=ALU.mult)
                zb = xp.tile([128, C], BF16, tag="l1zo")
                nc.vector.tensor_tensor(zb[:], z1[:], b1_bc[:], op=ALU.add)
                for g in range(4):
                    pt = trp.tile([128, 128], BF16)
                    nc.tensor.transpose(pt[:], zb[:, g * 128:(g + 1) * 128], ident[:])
                    nc.scalar.copy(zT[:, g, sl], pt[:])
                for w_sb, bias_sb, dst in (
                    (wq_sb, bq_sb, None),
                    (wk_sb, bk_sb, 0),
                    (wv_sb, bv_sb, 1),
                ):
                    ps = qkvp.tile([128, C], F32)
                    for g in range(4):
                        nc.tensor.matmul(ps[:], lhsT=zT[:, g, sl], rhs=w_sb[:, g, :],
                                         start=(g == 0), stop=False)
                    nc.tensor.matmul(ps[:], lhsT=ones_k1[:], rhs=bias_sb[:],
                                     start=False, stop=True)
                    if dst is None:
                        nc.scalar.copy(q_sb[:, ib, :], ps[:])
                    else:
                        ob = obp.tile([128, C], BF16)
                        nc.scalar.copy(ob[:], ps[:])
                        nc.sync.dma_start(out=kv_sh[sl, dst * C:(dst + 1) * C], in_=ob[:])
                # fire allgather for each finished quarter
                if (ib + 1) % BPA == 0:
                    j = (ib + 1) // BPA - 1
                    nc.gpsimd.collective_compute(
                        "AllGather", ALU.bypass,
                        replica_groups=[list(range(NCORES))],
                        ins=[kv_sh[j * LSH4:(j + 1) * LSH4, :]],
                        outs=[kv_full[j * NCORES * LSH4:(j + 1) * NCORES * LSH4, :]],
                    )

        # ---------------- phase E: edges + fused per-block tail ----------
        # Software-pipelined emission: per iteration ch we emit
        #   stage1(ch):  qps matmuls + prods + reduce + bias        [PE/DVE]
        #   wt(ch-1)                                                [DVE]
        #   dma(ch+1) prefetch                                      [Sync/Q7]
        #   acts(ch):   exp8 + expand                               [Act]
        #   scatter(ch-1) + block tails                             [PE/...]
        # so no engine stream head-of-line-blocks on another engine's
        # freshly-queued work.
        with ExitStack() as pctx:
            kvp = pctx.enter_context(tc.tile_pool(name="kvp", bufs=3))
            idxp = pctx.enter_context(tc.tile_pool(name="idxp", bufs=3))
            bp = pctx.enter_context(tc.tile_pool(name="bp", bufs=2))
            ohp_ = pctx.enter_context(tc.tile_pool(name="ohp", bufs=4))
            ohtp = pctx.enter_context(tc.tile_pool(name="ohtp", bufs=2))
            workp = pctx.enter_context(tc.tile_pool(name="workp", bufs=1))
            work = pctx.enter_context(tc.tile_pool(name="work", bufs=2))
            work3 = pctx.enter_context(tc.tile_pool(name="work3", bufs=3))
            pop_ = pctx.enter_context(tc.tile_pool(name="pout", bufs=1, space="PSUM"))
            psp = pctx.enter_context(tc.tile_pool(name="pssum", bufs=1, space="PSUM"))
            mmp = pctx.enter_context(tc.tile_pool(name="mm512", bufs=5, space="PSUM"))
            trp2 = pctx.enter_context(tc.tile_pool(name="trp2", bufs=1, space="PSUM"))
            finp = pctx.enter_context(tc.tile_pool(name="finp", bufs=1))
            lnp2 = pctx.enter_context(tc.tile_pool(name="lnp2", bufs=2))

            def _block_tail(rb_, pout, pssum):
                # ---- fused block tail: att, Wo, residual, LN2, MLP ----
                sl = slice(rb_ * 128, (rb_ + 1) * 128)
                sm = finp.tile([128, H], F32, tag="sm")
                nc.vector.tensor_scalar(sm[:], pssum[:], 1e-30, None, op0=ALU.max)
                rec = finp.tile([128, H], F32, tag="rec")
                nc.vector.reciprocal(rec[:], sm[:])
                rexp = finp.tile([128, C], BF16, tag="rexp")
                rap = bass.AP(tensor=rec.tensor, offset=rec[:].offset,
                              ap=[rec[:].ap[0], [1, H], [0, D]])
                nc.scalar.activation(
                    rexp[:].rearrange("p (h d) -> p h d", h=H), rap, AF.Copy)
                att = finp.tile([128, C], BF16, tag="att")
                nc.vector.tensor_tensor(att[:], pout[:], rexp[:], op=ALU.mult)
                attT = finp.tile([128, 4, 128], BF16, tag="attT")
                for g in range(4):
                    pt = trp2.tile([128, 128], BF16)
                    nc.tensor.transpose(pt[:], att[:, g * 128:(g + 1) * 128], ident[:])
                    nc.scalar.copy(attT[:, g, :], pt[:])
                po = mmp.tile([128, C], F32, tag="mm")
                for g in range(4):
                    nc.tensor.matmul(po[:], lhsT=attT[:, g, :], rhs=wo_sb[:, g, :],
                                     start=(g == 0), stop=False)
                nc.tensor.matmul(po[:], lhsT=ones_k1[:], rhs=bo_sb[:],
                                 start=False, stop=True)
                xb2 = finp.tile([128, C], F32, tag="xb2")
                nc.sync.dma_start(out=xb2[:], in_=x_c[sl, :])
                x1t = finp.tile([128, C], F32, tag="x1t")
                nc.vector.tensor_tensor(x1t[:], po[:], xb2[:], op=ALU.add)
                # LN2 + MLP
                z2 = layernorm(finp, lnp2, x1t, g2_bc, b2_bc, "l2")
                z2T = finp.tile([128, 4, 128], BF16, tag="z2T")
                for g in range(4):
                    pt = trp2.tile([128, 128], BF16)
                    nc.tensor.transpose(pt[:], z2[:, g * 128:(g + 1) * 128], ident[:])
                    nc.scalar.copy(z2T[:, g, :], pt[:])
                hs = finp.tile([128, 8, 128], BF16, tag="hs")
                for half in range(2):
                    ph_t = mmp.tile([128, C], F32, tag="mm")
                    ph = ph_t[:].rearrange("p (a b) -> p a b", a=4)
                    for c4 in range(4):
                        chc = half * 4 + c4
                        csl = slice(chc * 128, (chc + 1) * 128)
                        for g in range(4):
                            nc.tensor.matmul(ph[:, c4, :], lhsT=w1_sb[:, g, csl],
                                             rhs=z2T[:, g, :], start=(g == 0), stop=False)
                        nc.tensor.matmul(ph[:, c4, :], lhsT=b1_sb[:, csl],
                                         rhs=ones_k1[:], start=False, stop=True)
                    nc.scalar.activation(hs[:, half * 4:(half + 1) * 4, :], ph[:, :, :], AF.Silu)
                py = mmp.tile([128, C], F32, tag="mm")
                for chc in range(8):
                    nc.tensor.matmul(py[:], lhsT=hs[:, chc, :], rhs=w2_sb[:, chc, :],
                                     start=(chc == 0), stop=False)
                nc.tensor.matmul(py[:], lhsT=ones_k1[:], rhs=b2_sb[:],
                                 start=False, stop=True)
                yt = finp.tile([128, C], F32, tag="yt")
                nc.vector.tensor_tensor(yt[:], py[:], x1t[:], op=ALU.add)
                nc.sync.dma_start(out=y_out[sl, :], in_=yt[:])

            state = {"pout": None, "pssum": None}
            stash = {}

            def _emit_dma(ch):
                tiles_c = min(CHUNK_T, NT - ch * CHUNK_T)
                n_idx = tiles_c * 128
                cidx = idxp.tile([128, CHUNK_T * 8], I16, tag="cidx")
                nc.sync.dma_start(out=cidx[:], in_=colw[ch * 128:(ch + 1) * 128, :])
                kvb = kvp.tile([128, CHUNK_T, 2 * C], BF16)
                nc.gpsimd.dma_gather(
                    out_ap=kvb[:, :tiles_c, :], in_ap=kv_full[0:maxrow[ch], :],
                    idxs_ap=cidx[:, :n_idx // 16],
                    num_idxs=n_idx, num_idxs_reg=n_idx, elem_size=2 * C,
                    single_packet=False)
                bia = bp.tile([128, CHUNK_T, H], BF16, tag="bia")
                nc.sync.dma_start(
                    out=bia[:, :tiles_c, :],
                    in_=biasP[ch, :, :tiles_c * H].rearrange(
                        "p (t h) -> p t h", h=H))
                ohc = ohp_.tile([128, CHUNK_T, 128], BF16, tag="oh")
                nc.sync.dma_start(
                    out=ohc[:, :tiles_c, :],
                    in_=ohP[ch, :, :tiles_c * 128].rearrange(
                        "p (t r) -> p t r", r=128))
                ohtc = ohtp.tile([128, CHUNK_T, 128], BF16, tag="ohT")
                nc.sync.dma_start(
                    out=ohtc[:, :tiles_c, :],
                    in_=ohTP[ch, :, :tiles_c * 128].rearrange(
                        "p (t e) -> p t e", e=128))
                return dict(tiles_c=tiles_c, kvb=kvb, bia=bia, ohc=ohc, ohtc=ohtc)

            def _emit_stage1(ch, dd):
                tc_ = dd["tiles_c"]
                prod = workp.tile([128, CHUNK_T, C], BF16, tag="prod")
                for slot in range(tc_):
                    t = ch * CHUNK_T + slot
                    rb = t // T_BLK
                    qps = mmp.tile([128, C], F32, tag="mm")
                    nc.tensor.matmul(qps[:], lhsT=dd["ohtc"][:, slot, :],
                                     rhs=q_sb[:, rb, :], start=True, stop=True)
                    nc.vector.tensor_tensor(prod[:, slot, :], dd["kvb"][:, slot, 0:C],
                                            qps[:], op=ALU.mult)
                sc = work.tile([128, CHUNK_T, H], F32, tag="sc")
                nc.vector.tensor_reduce(
                    sc[:, :tc_, :],
                    prod[:, :tc_, :].rearrange("p t (h d) -> p t h d", h=H),
                    axis=AX.X, op=ALU.add)
                sc2 = work.tile([128, CHUNK_T, H], F32, tag="sc2")
                nc.vector.tensor_tensor(sc2[:, :tc_, :], sc[:, :tc_, :],
                                        dd["bia"][:, :tc_, :], op=ALU.add)
                dd["sc2"] = sc2

            def _emit_acts(ch, dd):
                tc_ = dd["tiles_c"]
                sc2 = dd["sc2"]
                p8c = work3.tile([128, CHUNK_T, H], BF16, tag="p8")
                nc.scalar.activation(p8c[:, :tc_, :], sc2[:, :tc_, :], AF.Exp)
                dd["p8c"] = p8c
                wtc = work3.tile([128, CHUNK_T, C], BF16, tag="wt")
                pexp = work.tile([128, CHUNK_T, C], BF16, tag="pexp")
                s2 = sc2[:, :tc_, :]
                src_b = bass.AP(tensor=s2.tensor, offset=s2.offset,
                                ap=[s2.ap[0], s2.ap[1], s2.ap[2], [0, D]])
                nc.scalar.activation(
                    pexp[:, :tc_, :].rearrange("p t (h d) -> p t h d", h=H),
                    src_b, AF.Exp)
                dd["wtc"] = wtc
                dd["pexp"] = pexp

            def _emit_wt(ch, dd):
                tc_ = dd["tiles_c"]
                wtc = dd["wtc"]
                nc.vector.tensor_tensor(wtc[:, :tc_, 0:C], dd["kvb"][:, :tc_, C:2 * C],
                                        dd["pexp"][:, :tc_, :], op=ALU.mult)

            def _emit_scatter(ch, dd):
                for s in range(dd["tiles_c"]):
                    ts_ = ch * CHUNK_T + s
                    rb_, tb_ = divmod(ts_, T_BLK)
                    if tb_ == 0:
                        state["pout"] = pop_.tile([128, C], F32, tag="pout", name="pout")
                        state["pssum"] = psp.tile([128, H], F32, tag="pssum", name="pssum")
                    nc.tensor.matmul(state["pout"][:], lhsT=dd["ohc"][:, s, :],
                                     rhs=dd["wtc"][:, s, :],
                                     start=(tb_ == 0), stop=(tb_ == T_BLK - 1))
                    nc.tensor.matmul(state["pssum"][:], lhsT=dd["ohc"][:, s, :],
                                     rhs=dd["p8c"][:, s, :],
                                     start=(tb_ == 0), stop=(tb_ == T_BLK - 1))
                    if tb_ == T_BLK - 1:
                        _block_tail(rb_, state["pout"], state["pssum"])

            # 3-stage pipeline: stage1(ch) | wt(ch-1) | scatter(ch-2)
            stash[0] = _emit_dma(0)
            for ch in range(NCH):
                _emit_stage1(ch, stash[ch])
                if ch >= 1:
                    _emit_wt(ch - 1, stash[ch - 1])
                if ch + 1 < NCH:
                    stash[ch + 1] = _emit_dma(ch + 1)
                _emit_acts(ch, stash[ch])
                if ch >= 2:
                    _emit_scatter(ch - 2, stash[ch - 2])
                    del stash[ch - 2]
            _emit_wt(NCH - 1, stash[NCH - 1])
            _emit_scatter(NCH - 2, stash[NCH - 2])
            _emit_scatter(NCH - 1, stash[NCH - 1])

    nc.finalize()
    _split_multi_waits(nc)
    return nc


# --------------------------------------------------------------------------
# entry point
# --------------------------------------------------------------------------

def kernel(**inputs) -> np.ndarray:
    x = np.asarray(inputs["x"], np.float32)
    row = np.asarray(inputs["row_index"]).astype(np.int64)
    col = np.asarray(inputs["col_index"]).astype(np.int64)
    att_bias = np.asarray(inputs["att_bias"], np.float32)
    L = x.shape[0]
    LSH = L // NCORES

    T_BLK, NT, NCH, maxrow, cores = _preprocess_edges(L, row, col, att_bias)
    # quantize AG-dep bounds to allgather chunk granularity for caching
    S = max(1, L // max(1, _nag(L // NCORES // 128)))
    maxrow = [min(L, -(-m // S) * S) for m in maxrow]

    w = _prep_weights(inputs)

    key = (L, T_BLK, NT, NCH, tuple(maxrow))
    if key not in _prog_cache:
        _prog_cache[key] = _build_program(L, T_BLK, NT, NCH, maxrow)
    nc = _prog_cache[key]

    in_maps = []
    for c in range(NCORES):
        m = dict(w)
        m["x_c"] = np.ascontiguousarray(x[c * LSH:(c + 1) * LSH])
        m.update(cores[c])
        in_maps.append(m)

    global LAST_EXEC_NS, LAST_RESULTS
    res = run_bass_kernel_spmd(nc, in_maps, list(range(NCORES)), trace=TRACE)
    LAST_RESULTS = res
    LAST_EXEC_NS = res.exec_time_ns
    return np.concatenate([res.results[c]["y"] for c in range(NCORES)], axis=0)


# revision 53
# speedup vs baseline: 1.1988x; 1.0573x over previous
"""Trainium2 Bass kernel for a sparse-attention EncoderLayer.

Sharding: rows (L) split into 8 contiguous shards of L/8; each edge is owned
by the core that owns its destination row (row_index is sorted, so each
core's edges are a contiguous range).  Each core computes Q/K/V for its row
shard; K/V shards are AllGathered (bf16, in 4 overlapping chunks) so every
core holds the full K/V table in HBM; per-edge K/V and Q rows are fetched
with dma_gather.  Segment softmax runs without max-subtraction (scores are
bounded, exp cannot overflow in f32).  Per-edge one-hot row selectors are
precomputed on the host and DMA'd in; the alpha-weighted scatter and softmax
sums are one-hot PE matmuls accumulated in PSUM per 128-row block.  The
LN2+MLP tail is fused into the edge phase per finished block.

DVE diet relative to the first version: the one-hot build, the p-broadcast
expansion and the x1 HBM roundtrip are gone; per-edge math is batched per
gather chunk (one DVE op per chunk instead of per 128-edge tile).
"""

import math
import numpy as np
from contextlib import ExitStack

from ml_dtypes import bfloat16

import concourse.bass as bass
import concourse.mybir as mybir
import concourse.tile as tile
from concourse import bacc
from concourse.bass_utils import run_bass_kernel_spmd
from concourse.masks import make_identity

NCORES = 8
C, H, D, HID = 512, 8, 64, 1024
EPS = 1e-5
CHUNK_T = 8   # edge tiles (of 128 edges) per dma_gather chunk
NAG = 8       # allgather chunks
F32 = mybir.dt.float32
BF16 = mybir.dt.bfloat16
I16 = mybir.dt.int16
AF = mybir.ActivationFunctionType
ALU = mybir.AluOpType
AX = mybir.AxisListType

_prog_cache = {}
TRACE = False
LAST_EXEC_NS = None
LAST_RESULTS = None


# --------------------------------------------------------------------------
# host-side preprocessing
# --------------------------------------------------------------------------

def _nag(NBLK):
    return NAG if NBLK % NAG == 0 else 1


def _wrap_idx(idx):
    """[n] int -> [128, n//16] int16, wrapped (idx i at partition i%16,
    column i//16) and replicated across the 8 Q7 cores."""
    n = idx.shape[0]
    w = np.ascontiguousarray(idx.reshape(n // 16, 16).T).astype(np.int16)
    return np.tile(w, (8, 1))


def _preprocess_edges(L, row, col, att_bias):
    LSH = L // NCORES
    NBLK = LSH // 128
    bounds = np.searchsorted(row, np.arange(NCORES + 1) * LSH)

    per_core = []
    t_blk = 1
    for c in range(NCORES):
        e0, e1 = int(bounds[c]), int(bounds[c + 1])
        r = row[e0:e1] - c * LSH
        blk = r >> 7
        cnt = np.bincount(blk, minlength=NBLK)
        t_blk = max(t_blk, int(np.max((cnt + 127) // 128)) if len(cnt) else 1)
        per_core.append((e0, e1, r, blk, cnt))

    T_BLK = t_blk
    NT = NBLK * T_BLK
    NCH = (NT + CHUNK_T - 1) // CHUNK_T
    NTP = NCH * CHUNK_T
    LSH4 = LSH // _nag(NBLK)

    cores = []
    for c in range(NCORES):
        e0, e1, r, blk, cnt = per_core[c]
        ne = e1 - e0
        starts = np.zeros(NBLK, dtype=np.int64)
        np.cumsum(cnt[:-1], out=starts[1:])

        npad = NTP * 128
        # col: global node id -> kv_full row (allgather chunk-major layout)
        gcol = col[e0:e1]
        oc, loc = gcol // LSH, gcol % LSH
        kvrow = (loc // LSH4) * (NCORES * LSH4) + oc * LSH4 + (loc % LSH4)
        # order edges within each block by kv row: improves gather locality
        # and lets early chunks depend on only a prefix of the allgather
        perm = np.lexsort((kvrow, blk))
        blk_s = blk[perm]
        kvrow_s = kvrow[perm]
        idx_in_blk = np.arange(ne, dtype=np.int64) - starts[blk_s]
        dst = blk_s * (T_BLK * 128) + idx_in_blk

        colP = np.zeros(npad, dtype=np.int64)
        rlocP = np.zeros(npad, dtype=np.int64)
        biasP = np.full((npad, H), -30000.0, dtype=np.float32)
        colP[dst] = kvrow_s
        rlocP[dst] = r[perm] & 127
        biasP[dst] = att_bias[e0:e1][perm]
        # per-chunk upper bound on referenced kv rows (for partial AG deps)
        maxrow = colP.reshape(NCH, CHUNK_T * 128).max(axis=1) + 1

        colw = _wrap_idx(colP).reshape(128, NCH, CHUNK_T * 8).transpose(1, 0, 2)
        colw = colw.reshape(NCH * 128, CHUNK_T * 8)
        # one-hot row selector per edge, chunk-partition-major for contiguous
        # DMA: ohP[ch, e(part), t_in_ch, r] bf16 via the u16 bit trick.
        # oh[t, e, r]: edge-partition (scatter lhsT); only real edges are set.
        ohu = np.zeros((NTP * 128, 128), dtype=np.uint16)
        ohu[dst, rlocP[dst]] = 0x3F80  # bf16 1.0
        oh = (ohu.view(bfloat16).reshape(NCH, CHUNK_T, 128, 128)
              .transpose(0, 2, 1, 3).reshape(NCH, 128, CHUNK_T * 128))
        # ohT[t, r, e]: row-partition (q-gather lhsT); set for ALL padded
        # slots too (col 0 row 0) so no garbage — padded p is 0 via bias.
        e_in_t = np.arange(npad, dtype=np.int64) % 128
        ohTu = np.zeros((NTP * 128, 128), dtype=np.uint16)
        ohTu[(np.arange(npad) // 128) * 128 + rlocP, e_in_t] = 0x3F80
        ohT = (ohTu.view(bfloat16).reshape(NCH, CHUNK_T, 128, 128)
               .transpose(0, 2, 1, 3).reshape(NCH, 128, CHUNK_T * 128))
        # bias, chunk-partition-major bf16: [NCH, 128, CHUNK_T*H]
        biasT = (biasP.reshape(NCH, CHUNK_T, 128, H).transpose(0, 2, 1, 3)
                 .reshape(NCH, 128, CHUNK_T * H).astype(bfloat16))
        cores.append(dict(
            colw=np.ascontiguousarray(colw),
            biasP=np.ascontiguousarray(biasT),
            ohP=np.ascontiguousarray(oh),
            ohTP=np.ascontiguousarray(ohT),
        ))
        cores[-1]["_maxrow"] = maxrow
    # chunk AG-dep bound must be identical across cores (same program):
    maxrow_all = np.max([c.pop("_maxrow") for c in cores], axis=0)
    return T_BLK, NT, NCH, [int(x) for x in maxrow_all], cores


def _prep_weights(inp):
    scale = 1.0 / math.sqrt(D)

    def mat(w, kchunks):
        w = np.asarray(w, np.float32)
        k, n = w.shape
        assert k == kchunks * 128
        return np.ascontiguousarray(
            w.reshape(kchunks, 128, n).transpose(1, 0, 2)).astype(bfloat16)

    def rowv(b):
        return np.asarray(b, np.float32)[None, :].astype(bfloat16)

    return dict(
        wq=mat(np.asarray(inp["Wq"], np.float32) * scale, 4),
        wk=mat(inp["Wk"], 4),
        wv=mat(inp["Wv"], 4),
        wo=mat(inp["Wo"], 4),
        w1=mat(inp["W1"], 4),
        w2=mat(inp["W2"], 8),
        bq=rowv(np.asarray(inp["bq"], np.float32) * scale),
        bk=rowv(inp["bk"]), bv=rowv(inp["bv"]), bo=rowv(inp["bo"]),
        b1=rowv(inp["b1"]), b2=rowv(inp["b2"]),
        ln1g=np.asarray(inp["ln1_g"], np.float32)[None, :].astype(bfloat16),
        ln1b=np.asarray(inp["ln1_b"], np.float32)[None, :].astype(bfloat16),
        ln2g=np.asarray(inp["ln2_g"], np.float32)[None, :].astype(bfloat16),
        ln2b=np.asarray(inp["ln2_b"], np.float32)[None, :].astype(bfloat16),
    )


# --------------------------------------------------------------------------
# walrus workaround: split Drain instructions carrying >1 sem wait
# --------------------------------------------------------------------------

def _split_multi_waits(nc):
    nid = [0]
    for fn in nc.m.functions:
        for blk in fn.blocks:
            insts = blk.instructions
            i = 0
            while i < len(insts):
                inst = insts[i]
                si = inst.sync_info
                if (isinstance(inst, mybir.InstDrain)
                        and si is not None and si.on_wait and len(si.on_wait) > 1):
                    waits = list(si.on_wait)
                    nops = []
                    for w in waits[:-1]:
                        nid[0] += 1
                        nops.append(mybir.InstNoOp(
                            name=f"I-waitfix-{nid[0]}",
                            engine=inst.engine, ins=[], outs=[],
                            sync_info=mybir.SyncInfo(on_wait=[w], on_update=[]),
                        ))
                    inst.sync_info = mybir.SyncInfo(
                        on_wait=[waits[-1]], on_update=list(si.on_update))
                    insts[i:i] = nops
                    i += len(nops)
                i += 1


# --------------------------------------------------------------------------
# device program
# --------------------------------------------------------------------------

def _build_program(L, T_BLK, NT, NCH, maxrow):
    LSH = L // NCORES
    NBLK = LSH // 128
    nag = _nag(NBLK)
    LSH4 = LSH // nag
    BPA = NBLK // nag  # blocks per allgather chunk
    nc = bacc.Bacc(num_devices=NCORES)

    x_c = nc.declare_dram_parameter("x_c", [LSH, C], F32, isOutput=False)
    wq = nc.declare_dram_parameter("wq", [128, 4, C], BF16, isOutput=False)
    wk = nc.declare_dram_parameter("wk", [128, 4, C], BF16, isOutput=False)
    wv = nc.declare_dram_parameter("wv", [128, 4, C], BF16, isOutput=False)
    wo = nc.declare_dram_parameter("wo", [128, 4, C], BF16, isOutput=False)
    w1 = nc.declare_dram_parameter("w1", [128, 4, HID], BF16, isOutput=False)
    w2 = nc.declare_dram_parameter("w2", [128, 8, C], BF16, isOutput=False)
    bqp = nc.declare_dram_parameter("bq", [1, C], BF16, isOutput=False)
    bkp = nc.declare_dram_parameter("bk", [1, C], BF16, isOutput=False)
    bvp = nc.declare_dram_parameter("bv", [1, C], BF16, isOutput=False)
    bop = nc.declare_dram_parameter("bo", [1, C], BF16, isOutput=False)
    b1p = nc.declare_dram_parameter("b1", [1, HID], BF16, isOutput=False)
    b2p = nc.declare_dram_parameter("b2", [1, C], BF16, isOutput=False)
    ln1g = nc.declare_dram_parameter("ln1g", [1, C], BF16, isOutput=False)
    ln1b = nc.declare_dram_parameter("ln1b", [1, C], BF16, isOutput=False)
    ln2g = nc.declare_dram_parameter("ln2g", [1, C], BF16, isOutput=False)
    ln2b = nc.declare_dram_parameter("ln2b", [1, C], BF16, isOutput=False)
    colw = nc.declare_dram_parameter("colw", [NCH * 128, CHUNK_T * 8], I16, isOutput=False)
    biasP = nc.declare_dram_parameter("biasP", [NCH, 128, CHUNK_T * H], BF16, isOutput=False)
    ohP = nc.declare_dram_parameter("ohP", [NCH, 128, CHUNK_T * 128], BF16, isOutput=False)
    ohTP = nc.declare_dram_parameter("ohTP", [NCH, 128, CHUNK_T * 128], BF16, isOutput=False)
    y_out = nc.declare_dram_parameter("y", [LSH, C], F32, isOutput=True)

    with ExitStack() as ctx:
        tc = ctx.enter_context(tile.TileContext(nc))

        dram = ctx.enter_context(tc.tile_pool(name="dram", bufs=1, space="DRAM"))
        kv_sh = dram.tile([LSH, 2 * C], BF16)
        # chunk-major full table: [NAG][NCORES][LSH4]
        kv_full = dram.tile([NCORES * LSH, 2 * C], BF16)

        # ---------------- constants + weights ----------------
        consts = ctx.enter_context(tc.tile_pool(name="consts", bufs=1))
        ident = consts.tile([128, 128], BF16, tag="ident")
        make_identity(nc, ident[:])
        ones_k1 = consts.tile([1, 128], BF16, tag="ones")
        nc.vector.memset(ones_k1[:], 1.0)
        eps_t = consts.tile([128, 1], F32, tag="eps")
        nc.vector.memset(eps_t[:], EPS)

        def bcast_load(param, tag):
            t = consts.tile([128, C], BF16, tag=tag)
            ap = param[:]
            src = bass.AP(tensor=ap.tensor, offset=ap.offset,
                          ap=[[0, 128], [1, C]])
            nc.sync.dma_start(out=t[:], in_=src)
            return t

        g1_bc, b1_bc = bcast_load(ln1g, "g1"), bcast_load(ln1b, "b1")
        g2_bc, b2_bc = bcast_load(ln2g, "g2"), bcast_load(ln2b, "b2")

        wts = ctx.enter_context(tc.tile_pool(name="wts", bufs=1))

        def wload(p, shape, tag):
            t = wts.tile(shape, BF16, tag=tag)
            nc.sync.dma_start(out=t[:], in_=p[:])
            return t

        wq_sb = wload(wq, [128, 4, C], "wq"); wk_sb = wload(wk, [128, 4, C], "wk")
        wv_sb = wload(wv, [128, 4, C], "wv"); wo_sb = wload(wo, [128, 4, C], "wo")
        w1_sb = wload(w1, [128, 4, HID], "w1"); w2_sb = wload(w2, [128, 8, C], "w2")
        bq_sb = wload(bqp, [1, C], "bq"); bk_sb = wload(bkp, [1, C], "bk")
        bv_sb = wload(bvp, [1, C], "bv"); bo_sb = wload(bop, [1, C], "bo")
        b1_sb = wload(b1p, [1, HID], "bb1"); b2_sb = wload(b2p, [1, C], "bb2")

        # ---------------- LN helper (fused tensor_scalar) ----------------
        def layernorm(pool, lnpool, xb, g_bc, bb_bc, tagp):
            stats = lnpool.tile([128, 6], F32, tag=tagp + "st")
            nc.vector.bn_stats(stats[:], xb[:])
            mv = lnpool.tile([128, 2], F32, tag=tagp + "mv")
            nc.vector.bn_aggr(mv[:], stats[:])
            sd = lnpool.tile([128, 1], F32, tag=tagp + "sd")
            nc.scalar.activation(sd[:], mv[:, 1:2], AF.Sqrt, bias=eps_t[:])
            rstd = lnpool.tile([128, 1], F32, tag=tagp + "rs")
            nc.vector.reciprocal(rstd[:], sd[:])
            z0 = pool.tile([128, C], BF16, tag=tagp + "z0")
            nc.vector.tensor_scalar(z0[:], xb[:], mv[:, 0:1], rstd[:],
                                    op0=ALU.subtract, op1=ALU.mult)
            z1 = pool.tile([128, C], BF16, tag=tagp + "z1")
            nc.vector.tensor_tensor(z1[:], z0[:], g_bc[:], op=ALU.mult)
            zb = pool.tile([128, C], BF16, tag=tagp + "zo")
            nc.vector.tensor_tensor(zb[:], z1[:], bb_bc[:], op=ALU.add)
            return zb

        # q table lives in SBUF for the whole run: [128 rows, NBLK, C]
        qtab_pool = ctx.enter_context(tc.tile_pool(name="qtab", bufs=1))
        q_sb = qtab_pool.tile([128, NBLK, C], BF16)

        # ---------------- phase B: LN1, zT, QKV (+chunked allgather) -------
        # LN1 stats are computed in a first sweep (DVE-only, batched sqrt
        # and reciprocal) so the per-block emission has no DVE<->Act
        # ping-pong on its critical path.
        with ExitStack() as pctx:
            zT_pool = pctx.enter_context(tc.tile_pool(name="zT", bufs=1))
            zT = zT_pool.tile([128, 4, LSH], BF16)
            xp = pctx.enter_context(tc.tile_pool(name="xp", bufs=3))
            lnp = pctx.enter_context(tc.tile_pool(name="lnp", bufs=4))
            trp = pctx.enter_context(tc.tile_pool(name="trp", bufs=2, space="PSUM"))
            qkvp = pctx.enter_context(tc.tile_pool(name="qkvp", bufs=2, space="PSUM"))
            obp = pctx.enter_context(tc.tile_pool(name="obp", bufs=3))

            mvall = zT_pool.tile([128, NBLK, 2], F32, tag="mvall")
            for ib in range(NBLK):
                sl = slice(ib * 128, (ib + 1) * 128)
                xb = xp.tile([128, C], F32, tag="xin")
                nc.sync.dma_start(out=xb[:], in_=x_c[sl, :])
                stats = lnp.tile([128, 6], F32, tag="l1st")
                nc.vector.bn_stats(stats[:], xb[:])
                nc.vector.bn_aggr(mvall[:, ib, :], stats[:])
            sdall = zT_pool.tile([128, NBLK], F32, tag="sdall")
            nc.scalar.activation(sdall[:], mvall[:, :, 1], AF.Sqrt, bias=eps_t[:])
            rsall = zT_pool.tile([128, NBLK], F32, tag="rsall")
            nc.vector.reciprocal(rsall[:], sdall[:])

            for ib in range(NBLK):
                sl = slice(ib * 128, (ib + 1) * 128)
                xb = xp.tile([128, C], F32, tag="xin")
                nc.sync.dma_start(out=xb[:], in_=x_c[sl, :])
                z0 = xp.tile([128, C], BF16, tag="l1z0")
                nc.vector.tensor_scalar(z0[:], xb[:], mvall[:, ib, 0:1],
                                        rsall[:, ib:ib + 1],
                                        op0=ALU.subtract, op1=ALU.mult)
                z1 = xp.tile([128, C], BF16, tag="l1z1")
                nc.vector.tensor_tensor(z1[:], z0[:], g1_bc[:], op=ALU.mult)
                zb = xp.tile([128, C], BF16, tag="l1zo")
                nc.vector.tensor_tensor(zb[:], z1[:], b1_bc[:], op=ALU.add)
                for g in range(4):
                    pt = trp.tile([128, 128], BF16)
                    nc.tensor.transpose(pt[:], zb[:, g * 128:(g + 1) * 128], ident[:])
                    nc.scalar.copy(zT[:, g, sl], pt[:])
                for w_sb, bias_sb, dst in (
                    (wq_sb, bq_sb, None),
                    (wk_sb, bk_sb, 0),
                    (wv_sb, bv_sb, 1),
                ):
                    ps = qkvp.tile([128, C], F32)
                    for g in range(4):
                        nc.tensor.matmul(ps[:], lhsT=zT[:, g, sl], rhs=w_sb[:, g, :],
                                         start=(g == 0), stop=False)
                    nc.tensor.matmul(ps[:], lhsT=ones_k1[:], rhs=bias_sb[:],
                                     start=False, stop=True)
                    if dst is None:
                        nc.scalar.copy(q_sb[:, ib, :], ps[:])
                    else:
                        ob = obp.tile([128, C], BF16)
                        nc.scalar.copy(ob[:], ps[:])
                        nc.sync.dma_start(out=kv_sh[sl, dst * C:(dst + 1) * C], in_=ob[:])
                # fire allgather for each finished quarter
                if (ib + 1) % BPA == 0:
                    j = (ib + 1) // BPA - 1
                    nc.gpsimd.collective_compute(
                        "AllGather", ALU.bypass,
                        replica_groups=[list(range(NCORES))],
                        ins=[kv_sh[j * LSH4:(j + 1) * LSH4, :]],
                        outs=[kv_full[j * NCORES * LSH4:(j + 1) * NCORES * LSH4, :]],
                    )

        # ---------------- phase E: edges + fused per-block tail ----------
        # Software-pipelined emission: per iteration ch we emit
        #   stage1(ch):  qps matmuls + prods + reduce + bias        [PE/DVE]
        #   wt(ch-1)                                                [DVE]
        #   dma(ch+1) prefetch                                      [Sync/Q7]
        #   acts(ch):   exp8 + expand                               [Act]
        #   scatter(ch-1) + block tails                             [PE/...]
        # so no engine stream head-of-line-blocks on another engine's
        # freshly-queued work.
        with ExitStack() as pctx:
            kvp = pctx.enter_context(tc.tile_pool(name="kvp", bufs=4))
            idxp = pctx.enter_context(tc.tile_pool(name="idxp", bufs=3))
            bp = pctx.enter_context(tc.tile_pool(name="bp", bufs=2))
            ohp_ = pctx.enter_context(tc.tile_pool(name="ohp", bufs=4))
            ohtp = pctx.enter_context(tc.tile_pool(name="ohtp", bufs=2))
            workp = pctx.enter_context(tc.tile_pool(name="workp", bufs=1))
            work = pctx.enter_context(tc.tile_pool(name="work", bufs=2))
            work3 = pctx.enter_context(tc.tile_pool(name="work3", bufs=3))
            pop_ = pctx.enter_context(tc.tile_pool(name="pout", bufs=1, space="PSUM"))
            psp = pctx.enter_context(tc.tile_pool(name="pssum", bufs=1, space="PSUM"))
            mmp = pctx.enter_context(tc.tile_pool(name="mm512", bufs=5, space="PSUM"))
            trp2 = pctx.enter_context(tc.tile_pool(name="trp2", bufs=1, space="PSUM"))
            finp = pctx.enter_context(tc.tile_pool(name="finp", bufs=1))
            lnp2 = pctx.enter_context(tc.tile_pool(name="lnp2", bufs=2))

            def _block_tail(rb_, pout, pssum):
                # ---- fused block tail: att, Wo, residual, LN2, MLP ----
                sl = slice(rb_ * 128, (rb_ + 1) * 128)
                sm = finp.tile([128, H], F32, tag="sm")
                nc.vector.tensor_scalar(sm[:], pssum[:], 1e-30, None, op0=ALU.max)
                rec = finp.tile([128, H], F32, tag="rec")
                nc.vector.reciprocal(rec[:], sm[:])
                rexp = finp.tile([128, C], BF16, tag="rexp")
                rap = bass.AP(tensor=rec.tensor, offset=rec[:].offset,
                              ap=[rec[:].ap[0], [1, H], [0, D]])
                nc.scalar.activation(
                    rexp[:].rearrange("p (h d) -> p h d", h=H), rap, AF.Copy)
                att = finp.tile([128, C], BF16, tag="att")
                nc.vector.tensor_tensor(att[:], pout[:], rexp[:], op=ALU.mult)
                attT = finp.tile([128, 4, 128], BF16, tag="attT")
                for g in range(4):
                    pt = trp2.tile([128, 128], BF16)
                    nc.tensor.transpose(pt[:], att[:, g * 128:(g + 1) * 128], ident[:])
                    nc.scalar.copy(attT[:, g, :], pt[:])
                po = mmp.tile([128, C], F32, tag="mm")
                for g in range(4):
                    nc.tensor.matmul(po[:], lhsT=attT[:, g, :], rhs=wo_sb[:, g, :],
                                     start=(g == 0), stop=False)
                nc.tensor.matmul(po[:], lhsT=ones_k1[:], rhs=bo_sb[:],
                                 start=False, stop=True)
                xb2 = finp.tile([128, C], F32, tag="xb2")
                nc.sync.dma_start(out=xb2[:], in_=x_c[sl, :])
                x1t = finp.tile([128, C], F32, tag="x1t")
                nc.vector.tensor_tensor(x1t[:], po[:], xb2[:], op=ALU.add)
                # LN2 + MLP
                z2 = layernorm(finp, lnp2, x1t, g2_bc, b2_bc, "l2")
                z2T = finp.tile([128, 4, 128], BF16, tag="z2T")
                for g in range(4):
                    pt = trp2.tile([128, 128], BF16)
                    nc.tensor.transpose(pt[:], z2[:, g * 128:(g + 1) * 128], ident[:])
                    nc.scalar.copy(z2T[:, g, :], pt[:])
                hs = finp.tile([128, 8, 128], BF16, tag="hs")
                for half in range(2):
                    ph_t = mmp.tile([128, C], F32, tag="mm")
                    ph = ph_t[:].rearrange("p (a b) -> p a b", a=4)
                    for c4 in range(4):
                        chc = half * 4 + c4
                        csl = slice(chc * 128, (chc + 1) * 128)
                        for g in range(4):
                            nc.tensor.matmul(ph[:, c4, :], lhsT=w1_sb[:, g, csl],
                                             rhs=z2T[:, g, :], start=(g == 0), stop=False)
                        nc.tensor.matmul(ph[:, c4, :], lhsT=b1_sb[:, csl],
                                         rhs=ones_k1[:], start=False, stop=True)
                    nc.scalar.activation(hs[:, half * 4:(half + 1) * 4, :], ph[:, :, :], AF.Silu)
                py = mmp.tile([128, C], F32, tag="mm")
                for chc in range(8):
                    nc.tensor.matmul(py[:], lhsT=hs[:, chc, :], rhs=w2_sb[:, chc, :],
                                     start=(chc == 0), stop=False)
                nc.tensor.matmul(py[:], lhsT=ones_k1[:], rhs=b2_sb[:],
                                 start=False, stop=True)
                yt = finp.tile([128, C], F32, tag="xb2", name="yt")
                nc.vector.tensor_tensor(yt[:], py[:], x1t[:], op=ALU.add)
                nc.sync.dma_start(out=y_out[sl, :], in_=yt[:])

            state = {"pout": None, "pssum": None}
            stash = {}

            def _emit_dma(ch):
                tiles_c = min(CHUNK_T, NT - ch * CHUNK_T)
                n_idx = tiles_c * 128
                cidx = idxp.tile([128, CHUNK_T * 8], I16, tag="cidx")
                nc.sync.dma_start(out=cidx[:], in_=colw[ch * 128:(ch + 1) * 128, :])
                kvb = kvp.tile([128, CHUNK_T, 2 * C], BF16)
                nc.gpsimd.dma_gather(
                    out_ap=kvb[:, :tiles_c, :], in_ap=kv_full[0:maxrow[ch], :],
                    idxs_ap=cidx[:, :n_idx // 16],
                    num_idxs=n_idx, num_idxs_reg=n_idx, elem_size=2 * C,
                    single_packet=False)
                bia = bp.tile([128, CHUNK_T, H], BF16, tag="bia")
                nc.sync.dma_start(
                    out=bia[:, :tiles_c, :],
                    in_=biasP[ch, :, :tiles_c * H].rearrange(
                        "p (t h) -> p t h", h=H))
                ohc = ohp_.tile([128, CHUNK_T, 128], BF16, tag="oh")
                nc.sync.dma_start(
                    out=ohc[:, :tiles_c, :],
                    in_=ohP[ch, :, :tiles_c * 128].rearrange(
                        "p (t r) -> p t r", r=128))
                ohtc = ohtp.tile([128, CHUNK_T, 128], BF16, tag="ohT")
                nc.sync.dma_start(
                    out=ohtc[:, :tiles_c, :],
                    in_=ohTP[ch, :, :tiles_c * 128].rearrange(
                        "p (t e) -> p t e", e=128))
                return dict(tiles_c=tiles_c, kvb=kvb, bia=bia, ohc=ohc, ohtc=ohtc)

            def _emit_stage1(ch, dd):
                tc_ = dd["tiles_c"]
                prod = workp.tile([128, CHUNK_T, C], BF16, tag="prod")
                for slot in range(tc_):
                    t = ch * CHUNK_T + slot
                    rb = t // T_BLK
                    qps = mmp.tile([128, C], F32, tag="mm")
                    nc.tensor.matmul(qps[:], lhsT=dd["ohtc"][:, slot, :],
                                     rhs=q_sb[:, rb, :], start=True, stop=True)
                    nc.vector.tensor_tensor(prod[:, slot, :], dd["kvb"][:, slot, 0:C],
                                            qps[:], op=ALU.mult)
                sc = work.tile([128, CHUNK_T, H], F32, tag="sc")
                nc.vector.tensor_reduce(
                    sc[:, :tc_, :],
                    prod[:, :tc_, :].rearrange("p t (h d) -> p t h d", h=H),
                    axis=AX.X, op=ALU.add)
                sc2 = work.tile([128, CHUNK_T, H], F32, tag="sc2")
                nc.vector.tensor_tensor(sc2[:, :tc_, :], sc[:, :tc_, :],
                                        dd["bia"][:, :tc_, :], op=ALU.add)
                dd["sc2"] = sc2

            def _emit_acts(ch, dd):
                tc_ = dd["tiles_c"]
                sc2 = dd["sc2"]
                p8c = work3.tile([128, CHUNK_T, H], BF16, tag="p8")
                nc.scalar.activation(p8c[:, :tc_, :], sc2[:, :tc_, :], AF.Exp)
                dd["p8c"] = p8c
                wtc = work3.tile([128, CHUNK_T, C], BF16, tag="wt")
                pexp = workp.tile([128, CHUNK_T, C], BF16, tag="pexp")
                s2 = sc2[:, :tc_, :]
                src_b = bass.AP(tensor=s2.tensor, offset=s2.offset,
                                ap=[s2.ap[0], s2.ap[1], s2.ap[2], [0, D]])
                nc.scalar.activation(
                    pexp[:, :tc_, :].rearrange("p t (h d) -> p t h d", h=H),
                    src_b, AF.Exp)
                dd["wtc"] = wtc
                dd["pexp"] = pexp

            def _emit_wt(ch, dd):
                tc_ = dd["tiles_c"]
                wtc = dd["wtc"]
                nc.vector.tensor_tensor(wtc[:, :tc_, 0:C], dd["kvb"][:, :tc_, C:2 * C],
                                        dd["pexp"][:, :tc_, :], op=ALU.mult)

            def _emit_scatter(ch, dd):
                for s in range(dd["tiles_c"]):
                    ts_ = ch * CHUNK_T + s
                    rb_, tb_ = divmod(ts_, T_BLK)
                    if tb_ == 0:
                        state["pout"] = pop_.tile([128, C], F32, tag="pout", name="pout")
                        state["pssum"] = psp.tile([128, H], F32, tag="pssum", name="pssum")
                    nc.tensor.matmul(state["pout"][:], lhsT=dd["ohc"][:, s, :],
                                     rhs=dd["wtc"][:, s, :],
                                     start=(tb_ == 0), stop=(tb_ == T_BLK - 1))
                    nc.tensor.matmul(state["pssum"][:], lhsT=dd["ohc"][:, s, :],
                                     rhs=dd["p8c"][:, s, :],
                                     start=(tb_ == 0), stop=(tb_ == T_BLK - 1))
                    if tb_ == T_BLK - 1:
                        _block_tail(rb_, state["pout"], state["pssum"])

            # 3-stage pipeline: stage1(ch) | wt(ch-1) | scatter(ch-2),
            # with gather DMAs prefetched two chunks ahead.
            stash[0] = _emit_dma(0)
            if NCH > 1:
                stash[1] = _emit_dma(1)
            for ch in range(NCH):
                _emit_stage1(ch, stash[ch])
                if ch >= 1:
                    _emit_wt(ch - 1, stash[ch - 1])
                if ch + 2 < NCH:
                    stash[ch + 2] = _emit_dma(ch + 2)
                _emit_acts(ch, stash[ch])
                if ch >= 2:
                    _emit_scatter(ch - 2, stash[ch - 2])
                    del stash[ch - 2]
            _emit_wt(NCH - 1, stash[NCH - 1])
            _emit_scatter(NCH - 2, stash[NCH - 2])
            _emit_scatter(NCH - 1, stash[NCH - 1])

    nc.finalize()
    _split_multi_waits(nc)
    return nc


# --------------------------------------------------------------------------
# entry point
# --------------------------------------------------------------------------

def kernel(**inputs) -> np.ndarray:
    x = np.asarray(inputs["x"], np.float32)
    row = np.asarray(inputs["row_index"]).astype(np.int64)
    col = np.asarray(inputs["col_index"]).astype(np.int64)
    att_bias = np.asarray(inputs["att_bias"], np.float32)
    L = x.shape[0]
    LSH = L // NCORES

    T_BLK, NT, NCH, maxrow, cores = _preprocess_edges(L, row, col, att_bias)
    # quantize AG-dep bounds to allgather chunk granularity for caching
    S = max(1, L // max(1, _nag(L // NCORES // 128)))
    maxrow = [min(L, -(-m // S) * S) for m in maxrow]

    w = _prep_weights(inputs)

    key = (L, T_BLK, NT, NCH, tuple(maxrow))
    if key not in _prog_cache:
        _prog_cache[key] = _build_program(L, T_BLK, NT, NCH, maxrow)
    nc = _prog_cache[key]

    in_maps = []
    for c in range(NCORES):
        m = dict(w)
        m["x_c"] = np.ascontiguousarray(x[c * LSH:(c + 1) * LSH])
        m.update(cores[c])
        in_maps.append(m)

    global LAST_EXEC_NS, LAST_RESULTS
    res = run_bass_kernel_spmd(nc, in_maps, list(range(NCORES)), trace=TRACE)
    LAST_RESULTS = res
    LAST_EXEC_NS = res.exec_time_ns
    return np.concatenate([res.results[c]["y"] for c in range(NCORES)], axis=0)


# revision 57
# speedup vs baseline: 1.3999x; 1.1678x over previous
"""Trainium2 Bass kernel for a sparse-attention EncoderLayer.

Sharding: rows (L) split into 8 contiguous shards of L/8; each edge is owned
by the core that owns its destination row (row_index is sorted, so each
core's edges are a contiguous range).  Each core computes Q/K/V for its row
shard; K/V shards are AllGathered (bf16, in 4 overlapping chunks) so every
core holds the full K/V table in HBM; per-edge K/V and Q rows are fetched
with dma_gather.  Segment softmax runs without max-subtraction (scores are
bounded, exp cannot overflow in f32).  Per-edge one-hot row selectors are
precomputed on the host and DMA'd in; the alpha-weighted scatter and softmax
sums are one-hot PE matmuls accumulated in PSUM per 128-row block.  The
LN2+MLP tail is fused into the edge phase per finished block.

DVE diet relative to the first version: the one-hot build, the p-broadcast
expansion and the x1 HBM roundtrip are gone; per-edge math is batched per
gather chunk (one DVE op per chunk instead of per 128-edge tile).
"""

import math
import numpy as np
from contextlib import ExitStack

from ml_dtypes import bfloat16

import concourse.bass as bass
import concourse.mybir as mybir
import concourse.tile as tile
from concourse import bacc
from concourse.bass_utils import run_bass_kernel_spmd
from concourse.masks import make_identity

NCORES = 8
C, H, D, HID = 512, 8, 64, 1024
EPS = 1e-5
CHUNK_T = 4   # edge tiles (of 128 edges) per dma_gather chunk
NAG = 8       # allgather chunks
F32 = mybir.dt.float32
BF16 = mybir.dt.bfloat16
I16 = mybir.dt.int16
AF = mybir.ActivationFunctionType
ALU = mybir.AluOpType
AX = mybir.AxisListType

_prog_cache = {}
TRACE = False
LAST_EXEC_NS = None
LAST_RESULTS = None


# --------------------------------------------------------------------------
# host-side preprocessing
# --------------------------------------------------------------------------

def _nag(NBLK):
    return NAG if NBLK % NAG == 0 else 1


def _wrap_idx(idx):
    """[n] int -> [128, n//16] int16, wrapped (idx i at partition i%16,
    column i//16) and replicated across the 8 Q7 cores."""
    n = idx.shape[0]
    w = np.ascontiguousarray(idx.reshape(n // 16, 16).T).astype(np.int16)
    return np.tile(w, (8, 1))


def _preprocess_edges(L, row, col, att_bias):
    LSH = L // NCORES
    NBLK = LSH // 128
    bounds = np.searchsorted(row, np.arange(NCORES + 1) * LSH)

    per_core = []
    t_blk = 1
    for c in range(NCORES):
        e0, e1 = int(bounds[c]), int(bounds[c + 1])
        r = row[e0:e1] - c * LSH
        blk = r >> 7
        cnt = np.bincount(blk, minlength=NBLK)
        t_blk = max(t_blk, int(np.max((cnt + 127) // 128)) if len(cnt) else 1)
        per_core.append((e0, e1, r, blk, cnt))

    T_BLK = t_blk
    NT = NBLK * T_BLK
    NCH = (NT + CHUNK_T - 1) // CHUNK_T
    NTP = NCH * CHUNK_T
    LSH4 = LSH // _nag(NBLK)

    cores = []
    for c in range(NCORES):
        e0, e1, r, blk, cnt = per_core[c]
        ne = e1 - e0
        starts = np.zeros(NBLK, dtype=np.int64)
        np.cumsum(cnt[:-1], out=starts[1:])

        npad = NTP * 128
        # col: global node id -> kv_full row (allgather chunk-major layout)
        gcol = col[e0:e1]
        oc, loc = gcol // LSH, gcol % LSH
        kvrow = (loc // LSH4) * (NCORES * LSH4) + oc * LSH4 + (loc % LSH4)
        # order edges within each block by kv row: improves gather locality
        # and lets early chunks depend on only a prefix of the allgather
        perm = np.lexsort((kvrow, blk))
        blk_s = blk[perm]
        kvrow_s = kvrow[perm]
        idx_in_blk = np.arange(ne, dtype=np.int64) - starts[blk_s]
        dst = blk_s * (T_BLK * 128) + idx_in_blk

        colP = np.zeros(npad, dtype=np.int64)
        rlocP = np.zeros(npad, dtype=np.int64)
        biasP = np.full((npad, H), -30000.0, dtype=np.float32)
        colP[dst] = kvrow_s
        rlocP[dst] = r[perm] & 127
        biasP[dst] = att_bias[e0:e1][perm]
        # per-chunk upper bound on referenced kv rows (for partial AG deps)
        maxrow = colP.reshape(NCH, CHUNK_T * 128).max(axis=1) + 1

        colw = _wrap_idx(colP).reshape(128, NCH, CHUNK_T * 8).transpose(1, 0, 2)
        colw = colw.reshape(NCH * 128, CHUNK_T * 8)
        # one-hot row selector per edge, chunk-partition-major for contiguous
        # DMA: ohP[ch, e(part), t_in_ch, r] bf16 via the u16 bit trick.
        # oh[t, e, r]: edge-partition (scatter lhsT); only real edges are set.
        ohu = np.zeros((NTP * 128, 128), dtype=np.uint16)
        ohu[dst, rlocP[dst]] = 0x3F80  # bf16 1.0
        oh = (ohu.view(bfloat16).reshape(NCH, CHUNK_T, 128, 128)
              .transpose(0, 2, 1, 3).reshape(NCH, 128, CHUNK_T * 128))
        # ohT[t, r, e]: row-partition (q-gather lhsT); set for ALL padded
        # slots too (col 0 row 0) so no garbage — padded p is 0 via bias.
        e_in_t = np.arange(npad, dtype=np.int64) % 128
        ohTu = np.zeros((NTP * 128, 128), dtype=np.uint16)
        ohTu[(np.arange(npad) // 128) * 128 + rlocP, e_in_t] = 0x3F80
        ohT = (ohTu.view(bfloat16).reshape(NCH, CHUNK_T, 128, 128)
               .transpose(0, 2, 1, 3).reshape(NCH, 128, CHUNK_T * 128))
        # bias, chunk-partition-major bf16: [NCH, 128, CHUNK_T*H]
        biasT = (biasP.reshape(NCH, CHUNK_T, 128, H).transpose(0, 2, 1, 3)
                 .reshape(NCH, 128, CHUNK_T * H).astype(bfloat16))
        cores.append(dict(
            colw=np.ascontiguousarray(colw),
            biasP=np.ascontiguousarray(biasT),
            ohP=np.ascontiguousarray(oh),
            ohTP=np.ascontiguousarray(ohT),
        ))
        cores[-1]["_maxrow"] = maxrow
    # chunk AG-dep bound must be identical across cores (same program):
    maxrow_all = np.max([c.pop("_maxrow") for c in cores], axis=0)
    return T_BLK, NT, NCH, [int(x) for x in maxrow_all], cores


def _prep_weights(inp):
    scale = 1.0 / math.sqrt(D)

    def mat(w, kchunks):
        w = np.asarray(w, np.float32)
        k, n = w.shape
        assert k == kchunks * 128
        return np.ascontiguousarray(
            w.reshape(kchunks, 128, n).transpose(1, 0, 2)).astype(bfloat16)

    def rowv(b):
        return np.asarray(b, np.float32)[None, :].astype(bfloat16)

    return dict(
        wq=mat(np.asarray(inp["Wq"], np.float32) * scale, 4),
        wk=mat(inp["Wk"], 4),
        wv=mat(inp["Wv"], 4),
        wo=mat(inp["Wo"], 4),
        w1=mat(inp["W1"], 4),
        w2=mat(inp["W2"], 8),
        bq=rowv(np.asarray(inp["bq"], np.float32) * scale),
        bk=rowv(inp["bk"]), bv=rowv(inp["bv"]), bo=rowv(inp["bo"]),
        b1=rowv(inp["b1"]), b2=rowv(inp["b2"]),
        ln1g=np.asarray(inp["ln1_g"], np.float32)[None, :].astype(bfloat16),
        ln1b=np.asarray(inp["ln1_b"], np.float32)[None, :].astype(bfloat16),
        ln2g=np.asarray(inp["ln2_g"], np.float32)[None, :].astype(bfloat16),
        ln2b=np.asarray(inp["ln2_b"], np.float32)[None, :].astype(bfloat16),
    )


# --------------------------------------------------------------------------
# walrus workaround: split Drain instructions carrying >1 sem wait
# --------------------------------------------------------------------------

def _split_multi_waits(nc):
    nid = [0]
    for fn in nc.m.functions:
        for blk in fn.blocks:
            insts = blk.instructions
            i = 0
            while i < len(insts):
                inst = insts[i]
                si = inst.sync_info
                if (isinstance(inst, mybir.InstDrain)
                        and si is not None and si.on_wait and len(si.on_wait) > 1):
                    waits = list(si.on_wait)
                    nops = []
                    for w in waits[:-1]:
                        nid[0] += 1
                        nops.append(mybir.InstNoOp(
                            name=f"I-waitfix-{nid[0]}",
                            engine=inst.engine, ins=[], outs=[],
                            sync_info=mybir.SyncInfo(on_wait=[w], on_update=[]),
                        ))
                    inst.sync_info = mybir.SyncInfo(
                        on_wait=[waits[-1]], on_update=list(si.on_update))
                    insts[i:i] = nops
                    i += len(nops)
                i += 1


# --------------------------------------------------------------------------
# device program
# --------------------------------------------------------------------------

def _build_program(L, T_BLK, NT, NCH, maxrow):
    LSH = L // NCORES
    NBLK = LSH // 128
    nag = _nag(NBLK)
    LSH4 = LSH // nag
    BPA = NBLK // nag  # blocks per allgather chunk
    nc = bacc.Bacc(num_devices=NCORES)

    x_c = nc.declare_dram_parameter("x_c", [LSH, C], F32, isOutput=False)
    wq = nc.declare_dram_parameter("wq", [128, 4, C], BF16, isOutput=False)
    wk = nc.declare_dram_parameter("wk", [128, 4, C], BF16, isOutput=False)
    wv = nc.declare_dram_parameter("wv", [128, 4, C], BF16, isOutput=False)
    wo = nc.declare_dram_parameter("wo", [128, 4, C], BF16, isOutput=False)
    w1 = nc.declare_dram_parameter("w1", [128, 4, HID], BF16, isOutput=False)
    w2 = nc.declare_dram_parameter("w2", [128, 8, C], BF16, isOutput=False)
    bqp = nc.declare_dram_parameter("bq", [1, C], BF16, isOutput=False)
    bkp = nc.declare_dram_parameter("bk", [1, C], BF16, isOutput=False)
    bvp = nc.declare_dram_parameter("bv", [1, C], BF16, isOutput=False)
    bop = nc.declare_dram_parameter("bo", [1, C], BF16, isOutput=False)
    b1p = nc.declare_dram_parameter("b1", [1, HID], BF16, isOutput=False)
    b2p = nc.declare_dram_parameter("b2", [1, C], BF16, isOutput=False)
    ln1g = nc.declare_dram_parameter("ln1g", [1, C], BF16, isOutput=False)
    ln1b = nc.declare_dram_parameter("ln1b", [1, C], BF16, isOutput=False)
    ln2g = nc.declare_dram_parameter("ln2g", [1, C], BF16, isOutput=False)
    ln2b = nc.declare_dram_parameter("ln2b", [1, C], BF16, isOutput=False)
    colw = nc.declare_dram_parameter("colw", [NCH * 128, CHUNK_T * 8], I16, isOutput=False)
    biasP = nc.declare_dram_parameter("biasP", [NCH, 128, CHUNK_T * H], BF16, isOutput=False)
    ohP = nc.declare_dram_parameter("ohP", [NCH, 128, CHUNK_T * 128], BF16, isOutput=False)
    ohTP = nc.declare_dram_parameter("ohTP", [NCH, 128, CHUNK_T * 128], BF16, isOutput=False)
    y_out = nc.declare_dram_parameter("y", [LSH, C], F32, isOutput=True)

    with ExitStack() as ctx:
        tc = ctx.enter_context(tile.TileContext(nc))

        dram = ctx.enter_context(tc.tile_pool(name="dram", bufs=1, space="DRAM"))
        kv_sh = dram.tile([LSH, 2 * C], BF16)
        # chunk-major full table: [NAG][NCORES][LSH4]
        kv_full = dram.tile([NCORES * LSH, 2 * C], BF16)

        # ---------------- constants + weights ----------------
        consts = ctx.enter_context(tc.tile_pool(name="consts", bufs=1))
        ident = consts.tile([128, 128], BF16, tag="ident")
        make_identity(nc, ident[:])
        ones_k1 = consts.tile([1, 128], BF16, tag="ones")
        nc.vector.memset(ones_k1[:], 1.0)
        eps_t = consts.tile([128, 1], F32, tag="eps")
        nc.vector.memset(eps_t[:], EPS)

        def bcast_load(param, tag):
            t = consts.tile([128, C], BF16, tag=tag)
            ap = param[:]
            src = bass.AP(tensor=ap.tensor, offset=ap.offset,
                          ap=[[0, 128], [1, C]])
            nc.sync.dma_start(out=t[:], in_=src)
            return t

        g1_bc, b1_bc = bcast_load(ln1g, "g1"), bcast_load(ln1b, "b1")
        g2_bc, b2_bc = bcast_load(ln2g, "g2"), bcast_load(ln2b, "b2")

        wts = ctx.enter_context(tc.tile_pool(name="wts", bufs=1))

        def wload(p, shape, tag):
            t = wts.tile(shape, BF16, tag=tag)
            nc.sync.dma_start(out=t[:], in_=p[:])
            return t

        wq_sb = wload(wq, [128, 4, C], "wq"); wk_sb = wload(wk, [128, 4, C], "wk")
        wv_sb = wload(wv, [128, 4, C], "wv"); wo_sb = wload(wo, [128, 4, C], "wo")
        w1_sb = wload(w1, [128, 4, HID], "w1"); w2_sb = wload(w2, [128, 8, C], "w2")
        bq_sb = wload(bqp, [1, C], "bq"); bk_sb = wload(bkp, [1, C], "bk")
        bv_sb = wload(bvp, [1, C], "bv"); bo_sb = wload(bop, [1, C], "bo")
        b1_sb = wload(b1p, [1, HID], "bb1"); b2_sb = wload(b2p, [1, C], "bb2")

        # ---------------- LN helper (fused tensor_scalar) ----------------
        def layernorm(pool, lnpool, xb, g_bc, bb_bc, tagp):
            stats = lnpool.tile([128, 6], F32, tag=tagp + "st")
            nc.vector.bn_stats(stats[:], xb[:])
            mv = lnpool.tile([128, 2], F32, tag=tagp + "mv")
            nc.vector.bn_aggr(mv[:], stats[:])
            sd = lnpool.tile([128, 1], F32, tag=tagp + "sd")
            nc.scalar.activation(sd[:], mv[:, 1:2], AF.Sqrt, bias=eps_t[:])
            rstd = lnpool.tile([128, 1], F32, tag=tagp + "rs")
            nc.vector.reciprocal(rstd[:], sd[:])
            z0 = pool.tile([128, C], BF16, tag=tagp + "z0")
            nc.vector.tensor_scalar(z0[:], xb[:], mv[:, 0:1], rstd[:],
                                    op0=ALU.subtract, op1=ALU.mult)
            z1 = pool.tile([128, C], BF16, tag=tagp + "z1")
            nc.vector.tensor_tensor(z1[:], z0[:], g_bc[:], op=ALU.mult)
            zb = pool.tile([128, C], BF16, tag=tagp + "zo")
            nc.vector.tensor_tensor(zb[:], z1[:], bb_bc[:], op=ALU.add)
            return zb

        # q table lives in SBUF for the whole run: [128 rows, NBLK, C]
        qtab_pool = ctx.enter_context(tc.tile_pool(name="qtab", bufs=1))
        q_sb = qtab_pool.tile([128, NBLK, C], BF16)

        # ---------------- phase B: LN1, zT, QKV (+chunked allgather) -------
        # LN1 stats are computed in a first sweep (DVE-only, batched sqrt
        # and reciprocal) so the per-block emission has no DVE<->Act
        # ping-pong on its critical path.
        with ExitStack() as pctx:
            zT_pool = pctx.enter_context(tc.tile_pool(name="zT", bufs=1))
            zT = zT_pool.tile([128, 4, LSH], BF16)
            xp = pctx.enter_context(tc.tile_pool(name="xp", bufs=3))
            lnp = pctx.enter_context(tc.tile_pool(name="lnp", bufs=4))
            trp = pctx.enter_context(tc.tile_pool(name="trp", bufs=2, space="PSUM"))
            qkvp = pctx.enter_context(tc.tile_pool(name="qkvp", bufs=2, space="PSUM"))
            obp = pctx.enter_context(tc.tile_pool(name="obp", bufs=3))

            mvall = zT_pool.tile([128, NBLK, 2], F32, tag="mvall")
            for ib in range(NBLK):
                sl = slice(ib * 128, (ib + 1) * 128)
                xb = xp.tile([128, C], F32, tag="xin")
                nc.sync.dma_start(out=xb[:], in_=x_c[sl, :])
                stats = lnp.tile([128, 6], F32, tag="l1st")
                nc.vector.bn_stats(stats[:], xb[:])
                nc.vector.bn_aggr(mvall[:, ib, :], stats[:])
            sdall = zT_pool.tile([128, NBLK], F32, tag="sdall")
            nc.scalar.activation(sdall[:], mvall[:, :, 1], AF.Sqrt, bias=eps_t[:])
            rsall = zT_pool.tile([128, NBLK], F32, tag="rsall")
            nc.vector.reciprocal(rsall[:], sdall[:])

            for ib in range(NBLK):
                sl = slice(ib * 128, (ib + 1) * 128)
                xb = xp.tile([128, C], F32, tag="xin")
                nc.sync.dma_start(out=xb[:], in_=x_c[sl, :])
                z0 = xp.tile([128, C], BF16, tag="l1z0")
                nc.vector.tensor_scalar(z0[:], xb[:], mvall[:, ib, 0:1],
                                        rsall[:, ib:ib + 1],
                                        op0=ALU.subtract, op1=ALU.mult)
                z1 = xp.tile([128, C], BF16, tag="l1z1")
                nc.vector.tensor_tensor(z1[:], z0[:], g1_bc[:], op=ALU.mult)
                zb = xp.tile([128, C], BF16, tag="l1zo")
                nc.vector.tensor_tensor(zb[:], z1[:], b1_bc[:], op=ALU.add)
                for g in range(4):
                    pt = trp.tile([128, 128], BF16)
                    nc.tensor.transpose(pt[:], zb[:, g * 128:(g + 1) * 128], ident[:])
                    nc.scalar.copy(zT[:, g, sl], pt[:])
                for w_sb, bias_sb, dst in (
                    (wq_sb, bq_sb, None),
                    (wk_sb, bk_sb, 0),
                    (wv_sb, bv_sb, 1),
                ):
                    ps = qkvp.tile([128, C], F32)
                    for g in range(4):
                        nc.tensor.matmul(ps[:], lhsT=zT[:, g, sl], rhs=w_sb[:, g, :],
                                         start=(g == 0), stop=False)
                    nc.tensor.matmul(ps[:], lhsT=ones_k1[:], rhs=bias_sb[:],
                                     start=False, stop=True)
                    if dst is None:
                        nc.scalar.copy(q_sb[:, ib, :], ps[:])
                    else:
                        ob = obp.tile([128, C], BF16)
                        nc.scalar.copy(ob[:], ps[:])
                        nc.sync.dma_start(out=kv_sh[sl, dst * C:(dst + 1) * C], in_=ob[:])
                # fire allgather for each finished quarter
                if (ib + 1) % BPA == 0:
                    j = (ib + 1) // BPA - 1
                    nc.gpsimd.collective_compute(
                        "AllGather", ALU.bypass,
                        replica_groups=[list(range(NCORES))],
                        ins=[kv_sh[j * LSH4:(j + 1) * LSH4, :]],
                        outs=[kv_full[j * NCORES * LSH4:(j + 1) * NCORES * LSH4, :]],
                    )

        # ---------------- phase E: edges + fused per-block tail ----------
        # Software-pipelined emission: per iteration ch we emit
        #   stage1(ch):  qps matmuls + prods + reduce + bias        [PE/DVE]
        #   wt(ch-1)                                                [DVE]
        #   dma(ch+1) prefetch                                      [Sync/Q7]
        #   acts(ch):   exp8 + expand                               [Act]
        #   scatter(ch-1) + block tails                             [PE/...]
        # so no engine stream head-of-line-blocks on another engine's
        # freshly-queued work.
        with ExitStack() as pctx:
            kvp = pctx.enter_context(tc.tile_pool(name="kvp", bufs=5))
            idxp = pctx.enter_context(tc.tile_pool(name="idxp", bufs=5))
            bp = pctx.enter_context(tc.tile_pool(name="bp", bufs=4))
            ohp_ = pctx.enter_context(tc.tile_pool(name="ohp", bufs=6))
            ohtp = pctx.enter_context(tc.tile_pool(name="ohtp", bufs=4))
            workp = pctx.enter_context(tc.tile_pool(name="workp", bufs=1))
            work = pctx.enter_context(tc.tile_pool(name="work", bufs=2))
            work3 = pctx.enter_context(tc.tile_pool(name="work3", bufs=3))
            pop_ = pctx.enter_context(tc.tile_pool(name="pout", bufs=1, space="PSUM"))
            psp = pctx.enter_context(tc.tile_pool(name="pssum", bufs=1, space="PSUM"))
            mmp = pctx.enter_context(tc.tile_pool(name="mm512", bufs=5, space="PSUM"))
            trp2 = pctx.enter_context(tc.tile_pool(name="trp2", bufs=1, space="PSUM"))
            finp = pctx.enter_context(tc.tile_pool(name="finp", bufs=1))
            lnp2 = pctx.enter_context(tc.tile_pool(name="lnp2", bufs=2))

            def _block_tail(rb_, pout, pssum):
                # ---- fused block tail: att, Wo, residual, LN2, MLP ----
                sl = slice(rb_ * 128, (rb_ + 1) * 128)
                sm = finp.tile([128, H], F32, tag="sm")
                nc.vector.tensor_scalar(sm[:], pssum[:], 1e-30, None, op0=ALU.max)
                rec = finp.tile([128, H], F32, tag="rec")
                nc.vector.reciprocal(rec[:], sm[:])
                rexp = finp.tile([128, C], BF16, tag="rexp")
                rap = bass.AP(tensor=rec.tensor, offset=rec[:].offset,
                              ap=[rec[:].ap[0], [1, H], [0, D]])
                nc.scalar.activation(
                    rexp[:].rearrange("p (h d) -> p h d", h=H), rap, AF.Copy)
                att = finp.tile([128, C], BF16, tag="att")
                nc.vector.tensor_tensor(att[:], pout[:], rexp[:], op=ALU.mult)
                attT = finp.tile([128, 4, 128], BF16, tag="attT")
                for g in range(4):
                    pt = trp2.tile([128, 128], BF16)
                    nc.tensor.transpose(pt[:], att[:, g * 128:(g + 1) * 128], ident[:])
                    nc.scalar.copy(attT[:, g, :], pt[:])
                po = mmp.tile([128, C], F32, tag="mm")
                for g in range(4):
                    nc.tensor.matmul(po[:], lhsT=attT[:, g, :], rhs=wo_sb[:, g, :],
                                     start=(g == 0), stop=False)
                nc.tensor.matmul(po[:], lhsT=ones_k1[:], rhs=bo_sb[:],
                                 start=False, stop=True)
                xb2 = finp.tile([128, C], F32, tag="xb2")
                nc.sync.dma_start(out=xb2[:], in_=x_c[sl, :])
                x1t = finp.tile([128, C], F32, tag="x1t")
                nc.vector.tensor_tensor(x1t[:], po[:], xb2[:], op=ALU.add)
                # LN2 + MLP
                z2 = layernorm(finp, lnp2, x1t, g2_bc, b2_bc, "l2")
                z2T = finp.tile([128, 4, 128], BF16, tag="z2T")
                for g in range(4):
                    pt = trp2.tile([128, 128], BF16)
                    nc.tensor.transpose(pt[:], z2[:, g * 128:(g + 1) * 128], ident[:])
                    nc.scalar.copy(z2T[:, g, :], pt[:])
                hs = finp.tile([128, 8, 128], BF16, tag="hs")
                for half in range(2):
                    ph_t = mmp.tile([128, C], F32, tag="mm")
                    ph = ph_t[:].rearrange("p (a b) -> p a b", a=4)
                    for c4 in range(4):
                        chc = half * 4 + c4
                        csl = slice(chc * 128, (chc + 1) * 128)
                        for g in range(4):
                            nc.tensor.matmul(ph[:, c4, :], lhsT=w1_sb[:, g, csl],
                                             rhs=z2T[:, g, :], start=(g == 0), stop=False)
                        nc.tensor.matmul(ph[:, c4, :], lhsT=b1_sb[:, csl],
                                         rhs=ones_k1[:], start=False, stop=True)
                    nc.scalar.activation(hs[:, half * 4:(half + 1) * 4, :], ph[:, :, :], AF.Silu)
                py = mmp.tile([128, C], F32, tag="mm")
                for chc in range(8):
                    nc.tensor.matmul(py[:], lhsT=hs[:, chc, :], rhs=w2_sb[:, chc, :],
                                     start=(chc == 0), stop=False)
                nc.tensor.matmul(py[:], lhsT=ones_k1[:], rhs=b2_sb[:],
                                 start=False, stop=True)
                yt = finp.tile([128, C], F32, tag="xb2", name="yt")
                nc.vector.tensor_tensor(yt[:], py[:], x1t[:], op=ALU.add)
                nc.sync.dma_start(out=y_out[sl, :], in_=yt[:])

            state = {"pout": None, "pssum": None}
            stash = {}

            def _emit_dma(ch):
                tiles_c = min(CHUNK_T, NT - ch * CHUNK_T)
                n_idx = tiles_c * 128
                cidx = idxp.tile([128, CHUNK_T * 8], I16, tag="cidx")
                nc.sync.dma_start(out=cidx[:], in_=colw[ch * 128:(ch + 1) * 128, :])
                kvb = kvp.tile([128, CHUNK_T, 2 * C], BF16)
                nc.gpsimd.dma_gather(
                    out_ap=kvb[:, :tiles_c, :], in_ap=kv_full[0:maxrow[ch], :],
                    idxs_ap=cidx[:, :n_idx // 16],
                    num_idxs=n_idx, num_idxs_reg=n_idx, elem_size=2 * C,
                    single_packet=False)
                bia = bp.tile([128, CHUNK_T, H], BF16, tag="bia")
                nc.sync.dma_start(
                    out=bia[:, :tiles_c, :],
                    in_=biasP[ch, :, :tiles_c * H].rearrange(
                        "p (t h) -> p t h", h=H))
                ohc = ohp_.tile([128, CHUNK_T, 128], BF16, tag="oh")
                nc.sync.dma_start(
                    out=ohc[:, :tiles_c, :],
                    in_=ohP[ch, :, :tiles_c * 128].rearrange(
                        "p (t r) -> p t r", r=128))
                ohtc = ohtp.tile([128, CHUNK_T, 128], BF16, tag="ohT")
                nc.sync.dma_start(
                    out=ohtc[:, :tiles_c, :],
                    in_=ohTP[ch, :, :tiles_c * 128].rearrange(
                        "p (t e) -> p t e", e=128))
                return dict(tiles_c=tiles_c, kvb=kvb, bia=bia, ohc=ohc, ohtc=ohtc)

            def _emit_stage1(ch, dd):
                tc_ = dd["tiles_c"]
                prod = workp.tile([128, CHUNK_T, C], BF16, tag="prod")
                qtiles = []
                for slot in range(tc_):
                    t = ch * CHUNK_T + slot
                    rb = t // T_BLK
                    qps = mmp.tile([128, C], F32, tag="mm")
                    nc.tensor.matmul(qps[:], lhsT=dd["ohtc"][:, slot, :],
                                     rhs=q_sb[:, rb, :], start=True, stop=True)
                    qtiles.append(qps)
                # even slots: Act copies PSUM->SBUF bf16 (cheap DVE read later);
                # odd slots: DVE reads PSUM f32 directly.
                qcp = work.tile([128, CHUNK_T // 2, C], BF16, tag="qcp")
                for j, slot in enumerate(range(0, tc_, 2)):
                    nc.scalar.copy(qcp[:, j, :], qtiles[slot][:])
                for slot in range(1, tc_, 2):
                    nc.vector.tensor_tensor(prod[:, slot, :], dd["kvb"][:, slot, 0:C],
                                            qtiles[slot][:], op=ALU.mult)
                for j, slot in enumerate(range(0, tc_, 2)):
                    nc.vector.tensor_tensor(prod[:, slot, :], dd["kvb"][:, slot, 0:C],
                                            qcp[:, j, :], op=ALU.mult)
                # tree reduce d: 64 -> 32 -> 16 -> 8, then axis-reduce
                pv = prod[:, :tc_, :].rearrange("p t (h d) -> p t h d", h=H)
                lv1 = workp.tile([128, CHUNK_T, H, 32], BF16, tag="lv1")
                nc.vector.tensor_tensor(lv1[:, :tc_, :, :], pv[:, :, :, 0:32],
                                        pv[:, :, :, 32:64], op=ALU.add)
                lv2 = workp.tile([128, CHUNK_T, H, 16], BF16, tag="lv2")
                nc.vector.tensor_tensor(lv2[:, :tc_, :, :], lv1[:, :tc_, :, 0:16],
                                        lv1[:, :tc_, :, 16:32], op=ALU.add)
                lv3 = workp.tile([128, CHUNK_T, H, 8], BF16, tag="lv3")
                nc.vector.tensor_tensor(lv3[:, :tc_, :, :], lv2[:, :tc_, :, 0:8],
                                        lv2[:, :tc_, :, 8:16], op=ALU.add)
                sc = work.tile([128, CHUNK_T, H], F32, tag="sc")
                nc.vector.tensor_reduce(sc[:, :tc_, :], lv3[:, :tc_, :, :],
                                        axis=AX.X, op=ALU.add)
                sc2 = work.tile([128, CHUNK_T, H], F32, tag="sc2")
                nc.vector.tensor_tensor(sc2[:, :tc_, :], sc[:, :tc_, :],
                                        dd["bia"][:, :tc_, :], op=ALU.add)
                dd["sc2"] = sc2

            def _emit_acts(ch, dd):
                tc_ = dd["tiles_c"]
                sc2 = dd["sc2"]
                p8c = work3.tile([128, CHUNK_T, H], BF16, tag="p8")
                nc.scalar.activation(p8c[:, :tc_, :], sc2[:, :tc_, :], AF.Exp)
                dd["p8c"] = p8c
                wtc = work3.tile([128, CHUNK_T, C], BF16, tag="wt")
                pexp = workp.tile([128, CHUNK_T, C], BF16, tag="pexp")
                s2 = sc2[:, :tc_, :]
                src_b = bass.AP(tensor=s2.tensor, offset=s2.offset,
                                ap=[s2.ap[0], s2.ap[1], s2.ap[2], [0, D]])
                nc.scalar.activation(
                    pexp[:, :tc_, :].rearrange("p t (h d) -> p t h d", h=H),
                    src_b, AF.Exp)
                dd["wtc"] = wtc
                dd["pexp"] = pexp

            def _emit_wt(ch, dd):
                tc_ = dd["tiles_c"]
                wtc = dd["wtc"]
                nc.vector.tensor_tensor(wtc[:, :tc_, 0:C], dd["kvb"][:, :tc_, C:2 * C],
                                        dd["pexp"][:, :tc_, :], op=ALU.mult)

            def _emit_scatter(ch, dd):
                for s in range(dd["tiles_c"]):
                    ts_ = ch * CHUNK_T + s
                    rb_, tb_ = divmod(ts_, T_BLK)
                    if tb_ == 0:
                        state["pout"] = pop_.tile([128, C], F32, tag="pout", name="pout")
                        state["pssum"] = psp.tile([128, H], F32, tag="pssum", name="pssum")
                    nc.tensor.matmul(state["pout"][:], lhsT=dd["ohc"][:, s, :],
                                     rhs=dd["wtc"][:, s, :],
                                     start=(tb_ == 0), stop=(tb_ == T_BLK - 1))
                    nc.tensor.matmul(state["pssum"][:], lhsT=dd["ohc"][:, s, :],
                                     rhs=dd["p8c"][:, s, :],
                                     start=(tb_ == 0), stop=(tb_ == T_BLK - 1))
                    if tb_ == T_BLK - 1:
                        _block_tail(rb_, state["pout"], state["pssum"])

            # 3-stage pipeline: stage1(ch) | wt(ch-1) | scatter(ch-2),
            # with gather DMAs prefetched three chunks ahead.
            LOOK = 3
            for j in range(min(LOOK, NCH)):
                stash[j] = _emit_dma(j)
            for ch in range(NCH):
                _emit_stage1(ch, stash[ch])
                if ch >= 1:
                    _emit_wt(ch - 1, stash[ch - 1])
                if ch + LOOK < NCH:
                    stash[ch + LOOK] = _emit_dma(ch + LOOK)
                _emit_acts(ch, stash[ch])
                if ch >= 2:
                    _emit_scatter(ch - 2, stash[ch - 2])
                    del stash[ch - 2]
            _emit_wt(NCH - 1, stash[NCH - 1])
            if NCH >= 2:
                _emit_scatter(NCH - 2, stash[NCH - 2])
            _emit_scatter(NCH - 1, stash[NCH - 1])

    nc.finalize()
    _split_multi_waits(nc)
    return nc


# --------------------------------------------------------------------------
# entry point
# --------------------------------------------------------------------------

def kernel(**inputs) -> np.ndarray:
    x = np.asarray(inputs["x"], np.float32)
    row = np.asarray(inputs["row_index"]).astype(np.int64)
    col = np.asarray(inputs["col_index"]).astype(np.int64)
    att_bias = np.asarray(inputs["att_bias"], np.float32)
    L = x.shape[0]
    LSH = L // NCORES

    T_BLK, NT, NCH, maxrow, cores = _preprocess_edges(L, row, col, att_bias)
    # quantize AG-dep bounds to allgather chunk granularity for caching
    S = max(1, L // max(1, _nag(L // NCORES // 128)))
    maxrow = [min(L, -(-m // S) * S) for m in maxrow]

    w = _prep_weights(inputs)

    key = (L, T_BLK, NT, NCH, tuple(maxrow))
    if key not in _prog_cache:
        _prog_cache[key] = _build_program(L, T_BLK, NT, NCH, maxrow)
    nc = _prog_cache[key]

    in_maps = []
    for c in range(NCORES):
        m = dict(w)
        m["x_c"] = np.ascontiguousarray(x[c * LSH:(c + 1) * LSH])
        m.update(cores[c])
        in_maps.append(m)

    global LAST_EXEC_NS, LAST_RESULTS
    res = run_bass_kernel_spmd(nc, in_maps, list(range(NCORES)), trace=TRACE)
    LAST_RESULTS = res
    LAST_EXEC_NS = res.exec_time_ns
    return np.concatenate([res.results[c]["y"] for c in range(NCORES)], axis=0)
